# revision 1
# baseline (speedup 1.0000x reference)
"""GQA with sliding-window + ALiBi (reduces to banded causal attention) on 8 TRN2 cores.

Sharding: 8 cores = 2 batches x 4 kv-head groups. Each core computes, for its
(batch b, kv group gi): Q projection for its 4 query heads, K/V projection for
its 1 kv head, banded sliding-window attention (window 1024, causal), and a
partial row-parallel Wo matmul. Host sums the 4 partials per batch.

Math notes (exact reductions of the reference):
- ALiBi bias is -clip(j-i,0)*slope: zero on all causal positions, nonzero only
  where the causal mask kills the score -> drop it entirely.
- The sliding mask adds +1.0 uniformly inside the window: softmax-invariant.
- Out-of-window/causal positions get -1e9 -> exp underflows to exactly 0.
- Scores are O(1), so softmax without max-subtraction is safe in fp32.
All matmuls run as float32r (measured bit-identical to fp32 on TRN2 HW, 4x rate).
"""
import math
from contextlib import ExitStack

import numpy as np

import concourse.tile as tile
from concourse import bacc, mybir
from concourse.bass_utils import run_bass_kernel_spmd
from concourse.masks import make_identity

dt = mybir.dt

B, S, H = 2, 2048, 2048
NUM_HEADS, KV_HEADS, D = 16, 4, 128
WINDOW = 1024
GH = 4            # query heads per kv head (per core)
GD = GH * D       # 512: per-core slice of the hidden dim
SCALE = 1.0 / math.sqrt(D)
NEG = -1e9
QB = 256          # query columns per attention group (2 blocks of 128)
NG = S // QB      # 8 query groups
KT = H // 128     # 16 contraction tiles for projections

_nc_cache = None


def _build_nc(ptp_bufs=2, hstp_bufs=2, vtp_bufs=2, gh_order='hg', phases=3):
    nc = bacc.Bacc()
    hsT = nc.declare_dram_parameter("hsT", [4, KT, 128, 512], dt.float32r, isOutput=False)
    wq = nc.declare_dram_parameter("wq", [H, GD], dt.float32r, isOutput=False)
    wk = nc.declare_dram_parameter("wk", [H, D], dt.float32r, isOutput=False)
    wv = nc.declare_dram_parameter("wv", [H, D], dt.float32r, isOutput=False)
    wo = nc.declare_dram_parameter("wo", [GD, H], dt.float32r, isOutput=False)
    masks = nc.declare_dram_parameter("masks", [4, 128, QB], dt.float32, isOutput=False)
    out = nc.declare_dram_parameter("out", [16, 4, 128, 512], dt.float32, isOutput=True)

    with tile.TileContext(nc) as tc, ExitStack() as ctx:
        consts = ctx.enter_context(tc.tile_pool(name="consts", bufs=1))
        wpool = ctx.enter_context(tc.tile_pool(name="wpool", bufs=1))
        big = ctx.enter_context(tc.tile_pool(name="big", bufs=1))
        hstp = ctx.enter_context(tc.tile_pool(name="hstp", bufs=hstp_bufs))
        vtp = ctx.enter_context(tc.tile_pool(name="vtp", bufs=vtp_bufs))
        ptp = ctx.enter_context(tc.tile_pool(name="ptp", bufs=ptp_bufs))
        smalls = ctx.enter_context(tc.tile_pool(name="smalls", bufs=4))
        outp = ctx.enter_context(tc.tile_pool(name="outp", bufs=4))
        psum = ctx.enter_context(tc.tile_pool(name="psum", bufs=8, space="PSUM"))

        # constants
        ident32 = consts.tile([128, 128], dt.float32)
        make_identity(nc, ident32)
        ident = consts.tile([128, 128], dt.float32r)
        nc.vector.tensor_copy(ident, ident32)
        ones32 = consts.tile([128, 128], dt.float32)
        nc.vector.memset(ones32, 1.0)
        ones = consts.tile([128, 128], dt.float32r)
        nc.vector.tensor_copy(ones, ones32)
        # weights: tiles declared here, DMAs issued inside chunk-0 loop so the
        # queue order interleaves weights with the first hst tiles
        wq_t = [wpool.tile([128, GD], dt.float32r, tag=f"wq{t}", name=f"wq{t}")
                for t in range(KT)]
        wk_t = [wpool.tile([128, D], dt.float32r, tag=f"wk{t}", name=f"wk{t}")
                for t in range(KT)]
        wv_t = [wpool.tile([128, D], dt.float32r, tag=f"wv{t}", name=f"wv{t}")
                for t in range(KT)]
        # persistent activations
        qT = [big.tile([128, S], dt.float32r, tag=f"qT{h}", name=f"qT{h}") for h in range(GH)]
        kT = big.tile([128, S], dt.float32r, tag="kT")
        v = big.tile([128, S], dt.float32r, tag="v")
        ohT = [big.tile([128, S], dt.float32r, tag=f"ohT{h}", name=f"ohT{h}") for h in range(GH)]

        # ---- Phase 1: projections (per 512-wide s-chunk) ----
        for ch in range(4):
            q_ps = [psum.tile([128, 512], dt.float32, tag="ps", name=f"qps{ch}_{h}") for h in range(GH)]
            k_ps = psum.tile([128, 512], dt.float32, tag="ps")
            v_ps = psum.tile([128, 512], dt.float32, tag="ps")
            for t in range(KT):
                if ch == 0:
                    nc.sync.dma_start(out=wq_t[t], in_=wq[t * 128:(t + 1) * 128, :])
                    nc.sync.dma_start(out=wk_t[t], in_=wk[t * 128:(t + 1) * 128, :])
                    nc.sync.dma_start(out=wv_t[t], in_=wv[t * 128:(t + 1) * 128, :])
                hst = hstp.tile([128, 512], dt.float32r, tag="hst")
                nc.sync.dma_start(out=hst, in_=hsT[ch, t])
                st = (t == 0)
                sp = (t == KT - 1)
                for h in range(GH):
                    nc.tensor.matmul(q_ps[h], lhsT=wq_t[t][:, h * 128:(h + 1) * 128],
                                     rhs=hst, start=st, stop=sp)
                nc.tensor.matmul(k_ps, lhsT=wk_t[t], rhs=hst, start=st, stop=sp)
                nc.tensor.matmul(v_ps, lhsT=wv_t[t], rhs=hst, start=st, stop=sp)
            for h in range(GH):
                nc.vector.tensor_copy(qT[h][:, ch * 512:(ch + 1) * 512], q_ps[h])
            nc.vector.tensor_copy(kT[:, ch * 512:(ch + 1) * 512], k_ps)
            vt = vtp.tile([128, 512], dt.float32r, tag="vt")
            nc.vector.tensor_copy(vt, v_ps)
            for j in range(4):
                tp = psum.tile([128, 128], dt.float32r, tag="ps")
                nc.tensor.transpose(tp, vt[:, j * 128:(j + 1) * 128], ident)
                nc.scalar.copy(
                    v[:, (4 * ch + j) * 128:(4 * ch + j + 1) * 128], tp)

        # deferred loads: needed only from attention/Wo onward
        mask_t = []
        for i in range(4):
            mt = consts.tile([128, QB], dt.float32, tag=f"mask{i}", name=f"mask{i}")
            nc.sync.dma_start(out=mt, in_=masks[i])
            mask_t.append(mt)
        wo_t = []
        for ct in range(4):
            wot = wpool.tile([128, H], dt.float32r, tag=f"wo{ct}", name=f"wo{ct}")
            nc.sync.dma_start(out=wot, in_=wo[ct * 128:(ct + 1) * 128, :])
            wo_t.append(wot)

        # ---- Phase 2: banded attention, scores transposed (S^T[k, q]) ----
        if phases < 2:
            for st in range(16):
                nc.sync.dma_start(out=out[st], in_=kT[:, :H].bitcast(dt.float32).rearrange("p (e n) -> e p n", e=4))
        mask_for_o = {1: 1, 0: 0, -7: 3, -8: 2}
        hg_pairs = ([(h, g) for h in range(GH) for g in range(NG)]
                    if gh_order == 'hg' else
                    [(h, g) for g in range(NG) for h in range(GH)])
        if phases < 2:
            hg_pairs = []
        for h, g in hg_pairs:
            if True:
                kjs = list(range(max(0, 2 * g - 8), 2 * g + 2))
                av = psum.tile([128, QB], dt.float32, tag="ps")
                den = psum.tile([1, QB], dt.float32, tag="ps")
                batches = [kjs[i:i + 2] for i in range(0, len(kjs), 2)]
                for bi, bk in enumerate(batches):
                    sps = psum.tile([128, QB * len(bk)], dt.float32, tag="ps")
                    for idx, kj in enumerate(bk):
                        nc.tensor.matmul(
                            sps[:, idx * QB:(idx + 1) * QB],
                            lhsT=kT[:, kj * 128:(kj + 1) * 128],
                            rhs=qT[h][:, g * QB:(g + 1) * QB],
                            start=True, stop=True)
                        mi = mask_for_o.get(kj - 2 * g)
                        if mi is not None:
                            nc.vector.tensor_add(
                                sps[:, idx * QB:(idx + 1) * QB],
                                sps[:, idx * QB:(idx + 1) * QB], mask_t[mi])
                    pt = ptp.tile([128, QB * 2], dt.float32r, tag="pt")
                    nc.scalar.activation(
                        pt[:, :QB * len(bk)], sps,
                        mybir.ActivationFunctionType.Exp, scale=SCALE)
                    for idx, kj in enumerate(bk):
                        first = (bi == 0 and idx == 0)
                        last = (kj == kjs[-1])
                        nc.tensor.matmul(
                            den, lhsT=ones[:, 0:1],
                            rhs=pt[:, idx * QB:(idx + 1) * QB],
                            start=first, stop=last)
                        nc.tensor.matmul(
                            av, lhsT=v[:, kj * 128:(kj + 1) * 128],
                            rhs=pt[:, idx * QB:(idx + 1) * QB],
                            start=first, stop=last)
                rc = smalls.tile([1, QB], dt.float32r, tag="rc")
                with nc.allow_low_precision(reason="f32r is full fp32 bits"):
                    nc.vector.reciprocal(rc, den)
                bc = psum.tile([128, QB], dt.float32, tag="ps")
                nc.tensor.matmul(bc, lhsT=ones[0:1, :], rhs=rc, start=True, stop=True)
                bcs = smalls.tile([128, QB], dt.float32, tag="bcs")
                nc.scalar.copy(bcs, bc)
                nc.vector.tensor_mul(ohT[h][:, g * QB:(g + 1) * QB], av, bcs)

        # ---- Phase 3: partial Wo (row-parallel) ----
        for st in range(16 if phases >= 3 else 0):
            for e in range(4):
                wops = psum.tile([128, 512], dt.float32, tag="ps")
                for ct in range(4):
                    nc.tensor.matmul(
                        wops, lhsT=ohT[ct][:, st * 128:(st + 1) * 128],
                        rhs=wo_t[ct][:, e * 512:(e + 1) * 512],
                        start=(ct == 0), stop=(ct == 3))
                osb = outp.tile([128, 512], dt.float32, tag="osb")
                nc.scalar.copy(osb, wops)
                nc.sync.dma_start(out=out[st, e], in_=osb)
        if phases == 2:
            for st2 in range(4):
                nc.sync.dma_start(out=out[st2], in_=ohT[st2].bitcast(dt.float32).rearrange("p (e n) -> e p n", e=4))

    nc.compile()
    return nc


def _build_masks():
    kk = np.arange(128)[:, None]
    qq = np.arange(128)[None, :]
    diag = np.where(kk <= qq, 0.0, NEG).astype(np.float32)
    edge = np.where(kk >= qq, 0.0, NEG).astype(np.float32)
    full = np.full((128, 128), NEG, np.float32)
    none = np.zeros((128, 128), np.float32)
    return np.stack([
        np.hstack([diag, none]),   # o = 0
        np.hstack([full, diag]),   # o = +1
        np.hstack([edge, full]),   # o = -8
        np.hstack([none, edge]),   # o = -7
    ])


def kernel(hidden_states, Wq, Wk, Wv, Wo):
    global _nc_cache
    if _nc_cache is None:
        _nc_cache = _build_nc()
    nc = _nc_cache

    masks = _build_masks()
    hsT = []
    for b in range(B):
        ht = np.ascontiguousarray(hidden_states[b].T)                 # [H, S]
        t4 = ht.reshape(KT, 128, 4, 512).transpose(2, 0, 1, 3)        # [ch, t, 128, 512]
        hsT.append(np.ascontiguousarray(t4))
    in_maps = []
    for b in range(B):
        for gi in range(KV_HEADS):
            in_maps.append({
                "hsT": hsT[b],
                "wq": np.ascontiguousarray(Wq[:, gi * GD:(gi + 1) * GD]),
                "wk": np.ascontiguousarray(Wk[:, gi * D:(gi + 1) * D]),
                "wv": np.ascontiguousarray(Wv[:, gi * D:(gi + 1) * D]),
                "wo": np.ascontiguousarray(Wo[gi * GD:(gi + 1) * GD, :]),
                "masks": masks,
            })
    res = run_bass_kernel_spmd(nc, in_maps, list(range(8)))
    out = np.zeros((B, S, H), np.float32)
    for b in range(B):
        acc = None
        for gi in range(KV_HEADS):
            o = res.results[b * KV_HEADS + gi]["out"]
            acc = o.copy() if acc is None else acc + o
        out[b] = acc.transpose(0, 2, 1, 3).reshape(S, H)              # [16,4,128,512] -> [S,H]
    return out



# revision 16
# speedup vs baseline: 1.7170x; 1.7170x over previous
"""GQA with sliding-window + ALiBi (reduces to banded causal attention) on 8 TRN2 cores.

Sharding: 8 cores = 2 batches x 4 kv-head groups. Each core computes, for its
(batch b, kv group gi): Q projection for its 4 query heads, K/V projection for
its 1 kv head, banded sliding-window attention (window 1024, causal), and a
partial row-parallel Wo matmul. Host sums the 4 partials per batch.

Math notes (exact reductions of the reference):
- ALiBi bias is -clip(j-i,0)*slope: zero on all causal positions, nonzero only
  where the causal mask kills the score -> drop it entirely.
- The sliding mask adds +1.0 uniformly inside the window: softmax-invariant.
- Out-of-window/causal positions get -1e9 -> exp underflows to exactly 0.
- Scores are O(1), so softmax without max-subtraction is safe.

Implementation (v2):
- All matmul operands and DMA traffic in bf16 (PSUM accumulation stays fp32);
  rel-err gate is 2e-2, bf16 keeps us ~1e-2 or better.
- Whole hsT slab persists in SBUF (64KB/partition bf16), loaded once.
- V is produced pre-transposed ([s,d] layout) straight from the projection.
- 128-wide q blocks: only two mask patterns (causal diag, window edge),
  preloaded into PSUM by a PE matmul so score matmuls accumulate onto them.
- softmax denominator: ones[128,128]@pt accumulated alongside av in the same
  PSUM bank; one DVE divide produces oh = av/den.
- Attention is software-pipelined (scores -> exp -> av/den lag queue) and
  iterated qb-outer so each Wo row-tile issues as soon as its q block is done.
"""
import math
from contextlib import ExitStack

import numpy as np

import concourse.tile as tile
from concourse import bacc, mybir
from concourse.bass_utils import run_bass_kernel_spmd
from concourse.masks import make_identity

dt = mybir.dt

B, S, H = 2, 2048, 2048
NUM_HEADS, KV_HEADS, D = 16, 4, 128
WINDOW = 1024
GH = 4            # query heads per kv head (per core)
GD = GH * D       # 512: per-core slice of the hidden dim
SCALE = 1.0 / math.sqrt(D)
NEG = -1e9
NB = S // 128     # 16 128-wide blocks along s
KT = H // 128     # 16 contraction tiles for projections

_nc_cache = None


def _build_nc(pipe_depth=3, main_bufs=6, wo_bufs=2, pt_bufs=8, osb_bufs=4, debug=0):
    nc = bacc.Bacc()
    hsT = nc.declare_dram_parameter("hsT", [KT, 128, S], dt.bfloat16, isOutput=False)
    wq = nc.declare_dram_parameter("wq", [H, GD], dt.bfloat16, isOutput=False)
    wk = nc.declare_dram_parameter("wk", [H, D], dt.bfloat16, isOutput=False)
    wv = nc.declare_dram_parameter("wv", [H, D], dt.bfloat16, isOutput=False)
    wo = nc.declare_dram_parameter("wo", [GD, H], dt.bfloat16, isOutput=False)
    masks = nc.declare_dram_parameter("masks", [2, 128, 128], dt.bfloat16, isOutput=False)
    out = nc.declare_dram_parameter("out", [NB, 4, 128, 512], dt.bfloat16, isOutput=True)

    with tile.TileContext(nc) as tc, ExitStack() as ctx:
        consts = ctx.enter_context(tc.tile_pool(name="consts", bufs=1))
        wpool = ctx.enter_context(tc.tile_pool(name="wpool", bufs=1))
        big = ctx.enter_context(tc.tile_pool(name="big", bufs=1))
        ptp = ctx.enter_context(tc.tile_pool(name="ptp", bufs=pt_bufs))
        smalls = ctx.enter_context(tc.tile_pool(name="smalls", bufs=4))
        outp = ctx.enter_context(tc.tile_pool(name="outp", bufs=osb_bufs))
        psum = ctx.enter_context(tc.tile_pool(name="psum", bufs=main_bufs, space="PSUM"))
        wops_p = ctx.enter_context(tc.tile_pool(name="wops", bufs=wo_bufs, space="PSUM"))

        # constants
        ident32 = consts.tile([128, 128], dt.float32)
        make_identity(nc, ident32)
        ident = consts.tile([128, 128], dt.bfloat16)
        nc.vector.tensor_copy(ident, ident32)
        ones32 = consts.tile([128, 128], dt.float32)
        nc.vector.memset(ones32, 1.0)
        ones = consts.tile([128, 128], dt.bfloat16)
        nc.vector.tensor_copy(ones, ones32)

        # weights + the whole hsT slab persist in SBUF (all bf16)
        wq_t = [wpool.tile([128, GD], dt.bfloat16, tag=f"wq{t}", name=f"wq{t}")
                for t in range(KT)]
        wk_t = [wpool.tile([128, D], dt.bfloat16, tag=f"wk{t}", name=f"wk{t}")
                for t in range(KT)]
        wv_t = [wpool.tile([128, D], dt.bfloat16, tag=f"wv{t}", name=f"wv{t}")
                for t in range(KT)]
        hs_t = [wpool.tile([128, S], dt.bfloat16, tag=f"hs{t}", name=f"hs{t}")
                for t in range(KT)]
        mask_t = []
        for i in range(2):
            mt = consts.tile([128, 128], dt.bfloat16, tag=f"mask{i}", name=f"mask{i}")
            mask_t.append(mt)
        wo_t = [wpool.tile([128, H], dt.bfloat16, tag=f"wo{ct}", name=f"wo{ct}")
                for ct in range(4)]

        # persistent activations
        qT = [big.tile([128, S], dt.bfloat16, tag=f"qT{h}", name=f"qT{h}") for h in range(GH)]
        kT = big.tile([128, S], dt.bfloat16, tag="kT")
        v = big.tile([128, S], dt.bfloat16, tag="v")  # [s%128, (sblk, d)]
        ohT = [big.tile([128, S], dt.bfloat16, tag=f"ohT{h}", name=f"ohT{h}") for h in range(GH)]

        # ---- DMA issue: first chunk's hs tiles + qkv weights, then the rest ----
        for t in range(KT):
            nc.sync.dma_start(out=wq_t[t], in_=wq[t * 128:(t + 1) * 128, :])
            nc.sync.dma_start(out=wk_t[t], in_=wk[t * 128:(t + 1) * 128, :])
            nc.sync.dma_start(out=wv_t[t], in_=wv[t * 128:(t + 1) * 128, :])
            nc.sync.dma_start(out=hs_t[t][:, 0:512], in_=hsT[t, :, 0:512])
        for ch in range(1, 4):
            for t in range(KT):
                nc.sync.dma_start(out=hs_t[t][:, ch * 512:(ch + 1) * 512],
                                  in_=hsT[t, :, ch * 512:(ch + 1) * 512])
        for i in range(2):
            nc.sync.dma_start(out=mask_t[i], in_=masks[i])
        for ct in range(4):
            nc.sync.dma_start(out=wo_t[ct], in_=wo[ct * 128:(ct + 1) * 128, :])

        # ---- Phase 1: projections per 256-wide half-chunk (3 PSUM banks each) ----
        # bank qps2: [h0 | h1] halves; qps2b: [h2 | h3]; bank kv: [k | v0 | v1]
        # NOTE: matmul start=True clears accumulation state for the WHOLE PSUM
        # bank, so co-resident groups in one bank must be issued contiguously
        # (a group fully closes before the next group's start): t-inner loops.
        for hc in range(8):
            s0 = hc * 256
            qps_a = psum.tile([128, 512], dt.float32, tag="ps", name=f"qa{hc}")
            qps_b = psum.tile([128, 512], dt.float32, tag="ps", name=f"qb{hc}")
            kv_ps = psum.tile([128, 512], dt.float32, tag="ps", name=f"kv{hc}")
            for h in range(2):
                for t in range(KT):
                    nc.tensor.matmul(qps_a[:, h * 256:(h + 1) * 256],
                                     lhsT=wq_t[t][:, h * 128:(h + 1) * 128],
                                     rhs=hs_t[t][:, s0:s0 + 256],
                                     start=(t == 0), stop=(t == KT - 1))
            for h in range(2):
                for t in range(KT):
                    nc.tensor.matmul(qps_b[:, h * 256:(h + 1) * 256],
                                     lhsT=wq_t[t][:, (h + 2) * 128:(h + 3) * 128],
                                     rhs=hs_t[t][:, s0:s0 + 256],
                                     start=(t == 0), stop=(t == KT - 1))
            for t in range(KT):
                nc.tensor.matmul(kv_ps[:, 0:256], lhsT=wk_t[t],
                                 rhs=hs_t[t][:, s0:s0 + 256],
                                 start=(t == 0), stop=(t == KT - 1))
            for j in range(2):
                for t in range(KT):
                    nc.tensor.matmul(kv_ps[:, 256 + j * 128:256 + (j + 1) * 128],
                                     lhsT=hs_t[t][:, s0 + j * 128:s0 + (j + 1) * 128],
                                     rhs=wv_t[t], start=(t == 0), stop=(t == KT - 1))
            for h in range(2):
                nc.vector.tensor_copy(qT[h][:, s0:s0 + 256],
                                      qps_a[:, h * 256:(h + 1) * 256])
                nc.scalar.copy(qT[h + 2][:, s0:s0 + 256],
                               qps_b[:, h * 256:(h + 1) * 256])
            nc.vector.tensor_copy(kT[:, s0:s0 + 256], kv_ps[:, 0:256])
            # v blocks 2*hc, 2*hc+1 -> v[:, blk*128:(blk+1)*128]
            nc.vector.tensor_copy(v[:, s0:s0 + 256], kv_ps[:, 256:512])

        if debug == 1:
            # dump projections: out[0..3]=qT, out[4]=kT, out[5]=v
            for e in range(4):
                for h in range(GH):
                    nc.sync.dma_start(out=out[h, e], in_=qT[h][:, e * 512:(e + 1) * 512])
                nc.sync.dma_start(out=out[4, e], in_=kT[:, e * 512:(e + 1) * 512])
                nc.sync.dma_start(out=out[5, e], in_=v[:, e * 512:(e + 1) * 512])

        # ---- Phase 2+3: banded attention (qb-outer) + Wo row-tiles ----
        # per (h, qb): kjs = [max(0, qb-8) .. qb]; score blocks [128k x 128q]
        # accumulated transposed; exp batches of <=4 blocks per PSUM bank.
        pending = []   # (avden, pts, pt, kj_list, first, last, h, qb)

        def flush_one():
            # av accumulates alone as the bank's open group; den is issued as
            # one contiguous group into the same bank only after av has closed
            # (a start=True clears accumulation bits bank-wide).
            avden, pts, pt, kjl, first, last, h, qb = pending.pop(0)
            n = len(kjl)
            for i, kj in enumerate(kjl):
                nc.tensor.matmul(avden[:, 0:128], lhsT=v[:, kj * 128:(kj + 1) * 128],
                                 rhs=pt[:, i * 128:(i + 1) * 128],
                                 start=(first and i == 0), stop=(last and i == n - 1))
            if last:
                nkj = sum(len(bk) for _, bk in pts)
                d = 0
                for ptt, bk in pts:
                    for i in range(len(bk)):
                        nc.tensor.matmul(avden[:, 128:256], lhsT=ones,
                                         rhs=ptt[:, i * 128:(i + 1) * 128],
                                         start=(d == 0), stop=(d == nkj - 1))
                        d += 1
                rcb = smalls.tile([128, 128], dt.float32, tag="rcb")
                with nc.allow_low_precision(reason="fp32 reciprocal, full precision"):
                    nc.vector.reciprocal(rcb, avden[:, 128:256])
                nc.vector.tensor_mul(ohT[h][:, qb * 128:(qb + 1) * 128],
                                     avden[:, 0:128], rcb)

        for qb in range(NB if debug != 1 else 0):
            for h in range(GH):
                kjs = list(range(max(0, qb - 8), qb + 1))
                avden = psum.tile([128, 512], dt.float32, tag="ps", name=f"ad{qb}_{h}")
                qs = qT[h][:, qb * 128:(qb + 1) * 128]
                pts = []
                for bi in range(0, len(kjs), 4):
                    bk = kjs[bi:bi + 4]
                    sps = psum.tile([128, 512], dt.float32, tag="ps")
                    for i, kj in enumerate(bk):
                        mi = 0 if kj == qb else (1 if kj == qb - 8 else None)
                        if mi is not None:
                            nc.tensor.matmul(sps[:, i * 128:(i + 1) * 128],
                                             lhsT=ident, rhs=mask_t[mi],
                                             start=True, stop=False)
                        nc.tensor.matmul(sps[:, i * 128:(i + 1) * 128],
                                         lhsT=kT[:, kj * 128:(kj + 1) * 128],
                                         rhs=qs, start=(mi is None), stop=True)
                    pt = ptp.tile([128, 512], dt.bfloat16, tag="pt")
                    nc.scalar.activation(pt[:, :128 * len(bk)], sps[:, :128 * len(bk)],
                                         mybir.ActivationFunctionType.Exp, scale=SCALE)
                    pts.append((pt, bk))
                    pending.append((avden, pts, pt, bk, bi == 0, bi + 4 >= len(kjs), h, qb))
                    while len(pending) > pipe_depth:
                        flush_one()
            # Wo row-tile st=qb-1 (lag one qb so divides have definitely issued)
            if qb >= 1:
                emit_wo(nc, wops_p, outp, ohT, wo_t, out, qb - 1)
        while pending:
            flush_one()
        if debug != 1:
            emit_wo(nc, wops_p, outp, ohT, wo_t, out, NB - 1)

    nc.compile()
    return nc


def emit_wo(nc, wops_p, outp, ohT, wo_t, out, st):
    for e in range(4):
        wops = wops_p.tile([128, 512], dt.float32, tag="wo")
        for ct in range(4):
            nc.tensor.matmul(wops, lhsT=ohT[ct][:, st * 128:(st + 1) * 128],
                             rhs=wo_t[ct][:, e * 512:(e + 1) * 512],
                             start=(ct == 0), stop=(ct == 3))
        osb = outp.tile([128, 512], dt.bfloat16, tag="osb")
        nc.vector.tensor_copy(osb, wops)
        nc.sync.dma_start(out=out[st, e], in_=osb)


def _build_masks():
    kk = np.arange(128)[:, None]
    qq = np.arange(128)[None, :]
    diag = np.where(kk <= qq, 0.0, NEG).astype(np.float32)   # causal within diag block
    edge = np.where(kk >= qq, 0.0, NEG).astype(np.float32)   # window lower edge
    return np.stack([diag, edge])


def kernel(hidden_states, Wq, Wk, Wv, Wo):
    global _nc_cache
    if _nc_cache is None:
        _nc_cache = _build_nc()
    nc = _nc_cache

    bf16 = dt.np(dt.bfloat16)
    masks = _build_masks().astype(bf16)
    hsT = []
    for b in range(B):
        ht = np.ascontiguousarray(hidden_states[b].T.astype(bf16))    # [H, S]
        hsT.append(np.ascontiguousarray(ht.reshape(KT, 128, S)))
    in_maps = []
    for b in range(B):
        for gi in range(KV_HEADS):
            in_maps.append({
                "hsT": hsT[b],
                "wq": np.ascontiguousarray(Wq[:, gi * GD:(gi + 1) * GD].astype(bf16)),
                "wk": np.ascontiguousarray(Wk[:, gi * D:(gi + 1) * D].astype(bf16)),
                "wv": np.ascontiguousarray(Wv[:, gi * D:(gi + 1) * D].astype(bf16)),
                "wo": np.ascontiguousarray(Wo[gi * GD:(gi + 1) * GD, :].astype(bf16)),
                "masks": masks,
            })
    res = run_bass_kernel_spmd(nc, in_maps, list(range(8)))
    out = np.zeros((B, S, H), np.float32)
    for b in range(B):
        acc = None
        for gi in range(KV_HEADS):
            o = res.results[b * KV_HEADS + gi]["out"].astype(np.float32)
            acc = o if acc is None else acc + o
        out[b] = acc.transpose(0, 2, 1, 3).reshape(S, H)              # [16,4,128,512] -> [S,H]
    return out


# revision 21
# speedup vs baseline: 1.8687x; 1.0884x over previous
"""GQA with sliding-window + ALiBi (reduces to banded causal attention) on 8 TRN2 cores.

Sharding: 8 cores = 2 batches x 4 kv-head groups. Each core computes, for its
(batch b, kv group gi): Q projection for its 4 query heads, K/V projection for
its 1 kv head, banded sliding-window attention (window 1024, causal), and a
partial row-parallel Wo matmul. Host sums the 4 partials per batch.

Math notes (exact reductions of the reference):
- ALiBi bias is -clip(j-i,0)*slope: zero on all causal positions, nonzero only
  where the causal mask kills the score -> drop it entirely.
- The sliding mask adds +1.0 uniformly inside the window: softmax-invariant.
- Out-of-window/causal positions get -1e9 -> exp underflows to exactly 0.
- Scores are O(1), so softmax without max-subtraction is safe.

Implementation (v2):
- All matmul operands and DMA traffic in bf16 (PSUM accumulation stays fp32);
  rel-err gate is 2e-2, bf16 keeps us ~1e-2 or better.
- Whole hsT slab persists in SBUF (64KB/partition bf16), loaded once.
- V is produced pre-transposed ([s,d] layout) straight from the projection.
- 128-wide q blocks: only two mask patterns (causal diag, window edge),
  preloaded into PSUM by a PE matmul so score matmuls accumulate onto them.
- softmax denominator: ones[128,128]@pt accumulated alongside av in the same
  PSUM bank; one DVE divide produces oh = av/den.
- Attention is software-pipelined (scores -> exp -> av/den lag queue) and
  iterated qb-outer so each Wo row-tile issues as soon as its q block is done.
"""
import math
from contextlib import ExitStack

import numpy as np

import concourse.tile as tile
from concourse import bacc, mybir
from concourse.bass_utils import run_bass_kernel_spmd
from concourse.masks import make_identity

dt = mybir.dt

B, S, H = 2, 2048, 2048
NUM_HEADS, KV_HEADS, D = 16, 4, 128
WINDOW = 1024
GH = 4            # query heads per kv head (per core)
GD = GH * D       # 512: per-core slice of the hidden dim
SCALE = 1.0 / math.sqrt(D)
NEG = -1e9
NB = S // 128     # 16 128-wide blocks along s
KT = H // 128     # 16 contraction tiles for projections

_nc_cache = None


def _build_nc(pipe_depth=3, main_bufs=6, wo_bufs=2, pt_bufs=8, osb_bufs=4, debug=0):
    nc = bacc.Bacc()
    hsT = nc.declare_dram_parameter("hsT", [KT, 128, S], dt.bfloat16, isOutput=False)
    wq = nc.declare_dram_parameter("wq", [H, GD], dt.bfloat16, isOutput=False)
    wk = nc.declare_dram_parameter("wk", [H, D], dt.bfloat16, isOutput=False)
    wv = nc.declare_dram_parameter("wv", [H, D], dt.bfloat16, isOutput=False)
    wo = nc.declare_dram_parameter("wo", [GD, H], dt.bfloat16, isOutput=False)
    masks = nc.declare_dram_parameter("masks", [2, 128, 128], dt.bfloat16, isOutput=False)
    out = nc.declare_dram_parameter("out", [NB, 4, 128, 512], dt.bfloat16, isOutput=True)

    with tile.TileContext(nc) as tc, ExitStack() as ctx:
        consts = ctx.enter_context(tc.tile_pool(name="consts", bufs=1))
        wpool = ctx.enter_context(tc.tile_pool(name="wpool", bufs=1))
        big = ctx.enter_context(tc.tile_pool(name="big", bufs=1))
        ptp = ctx.enter_context(tc.tile_pool(name="ptp", bufs=pt_bufs))
        smalls = ctx.enter_context(tc.tile_pool(name="smalls", bufs=4))
        outp = ctx.enter_context(tc.tile_pool(name="outp", bufs=osb_bufs))
        psum = ctx.enter_context(tc.tile_pool(name="psum", bufs=main_bufs, space="PSUM"))
        wops_p = ctx.enter_context(tc.tile_pool(name="wops", bufs=wo_bufs, space="PSUM"))

        # constants
        ident32 = consts.tile([128, 128], dt.float32)
        make_identity(nc, ident32)
        ident = consts.tile([128, 128], dt.bfloat16)
        nc.vector.tensor_copy(ident, ident32)
        ones32 = consts.tile([128, 128], dt.float32)
        nc.vector.memset(ones32, 1.0)
        ones = consts.tile([128, 128], dt.bfloat16)
        nc.vector.tensor_copy(ones, ones32)

        # weights + the whole hsT slab persist in SBUF (all bf16).
        # Few BIG multi-tile DMAs (HWDGE queue overhead is per-instruction).
        wq_all = wpool.tile([128, KT * GD], dt.bfloat16, tag="wq", name="wq_all")
        wk_all = wpool.tile([128, KT * D], dt.bfloat16, tag="wk", name="wk_all")
        wv_all = wpool.tile([128, KT * D], dt.bfloat16, tag="wv", name="wv_all")
        hs_all = wpool.tile([128, KT * S], dt.bfloat16, tag="hs", name="hs_all")
        wq_t = [wq_all[:, t * GD:(t + 1) * GD] for t in range(KT)]
        wk_t = [wk_all[:, t * D:(t + 1) * D] for t in range(KT)]
        wv_t = [wv_all[:, t * D:(t + 1) * D] for t in range(KT)]
        hs_t = [hs_all[:, t * S:(t + 1) * S] for t in range(KT)]
        mask_t = []
        for i in range(2):
            mt = consts.tile([128, 128], dt.bfloat16, tag=f"mask{i}", name=f"mask{i}")
            mask_t.append(mt)
        wo_all = wpool.tile([128, 4 * H], dt.bfloat16, tag="wo", name="wo_all")
        wo_t = [wo_all[:, ct * H:(ct + 1) * H] for ct in range(4)]

        # persistent activations
        qT = [big.tile([128, S], dt.bfloat16, tag=f"qT{h}", name=f"qT{h}") for h in range(GH)]
        kT = big.tile([128, S], dt.bfloat16, tag="kT")
        v = big.tile([128, S], dt.bfloat16, tag="v")  # [s%128, (sblk, d)]
        ohT = [big.tile([128, S], dt.bfloat16, tag=f"ohT{h}", name=f"ohT{h}") for h in range(GH)]

        # ---- DMA issue: first chunk's operands first, then the rest ----
        # wq as two halves (t 0-7, 8-15) so the first q-group starts sooner;
        # hs chunk DMAs carry all 16 t-tiles' 512-column slices in one instr.
        # SBUF-side DMA APs need the partition dim outermost; DRAM side is
        # rearranged to the same p-outer element order.
        def dma_hs_chunk(ch, t0, t1):
            nc.sync.dma_start(
                out=hs_all.rearrange("p (t s) -> p t s", t=KT)[:, t0:t1, ch * 512:(ch + 1) * 512],
                in_=hsT.rearrange("t p s -> p t s")[:, t0:t1, ch * 512:(ch + 1) * 512])

        wq_sb = wq_all.rearrange("p (t n) -> p t n", t=KT)
        wq_dr = wq.rearrange("(t p) n -> p t n", t=KT)
        nc.sync.dma_start(out=wq_sb[:, 0:8], in_=wq_dr[:, 0:8])
        dma_hs_chunk(0, 0, 8)
        nc.sync.dma_start(out=wq_sb[:, 8:16], in_=wq_dr[:, 8:16])
        dma_hs_chunk(0, 8, 16)
        nc.sync.dma_start(out=wk_all.rearrange("p (t n) -> p t n", t=KT),
                          in_=wk.rearrange("(t p) n -> p t n", t=KT))
        nc.sync.dma_start(out=wv_all.rearrange("p (t n) -> p t n", t=KT),
                          in_=wv.rearrange("(t p) n -> p t n", t=KT))
        for ch in range(1, 4):
            dma_hs_chunk(ch, 0, 16)
        for i in range(2):
            nc.sync.dma_start(out=mask_t[i], in_=masks[i])
        nc.sync.dma_start(out=wo_all.rearrange("p (t n) -> p t n", t=4),
                          in_=wo.rearrange("(t p) n -> p t n", t=4))

        # ---- Phase 1: projections per 256-wide half-chunk (3 PSUM banks each) ----
        # bank qps2: [h0 | h1] halves; qps2b: [h2 | h3]; bank kv: [k | v0 | v1]
        # NOTE: matmul start=True clears accumulation state for the WHOLE PSUM
        # bank, so co-resident groups in one bank must be issued contiguously
        # (a group fully closes before the next group's start): t-inner loops.
        for hc in range(8):
            s0 = hc * 256
            qps_a = psum.tile([128, 512], dt.float32, tag="ps", name=f"qa{hc}")
            qps_b = psum.tile([128, 512], dt.float32, tag="ps", name=f"qb{hc}")
            kv_ps = psum.tile([128, 512], dt.float32, tag="ps", name=f"kv{hc}")
            for h in range(2):
                for t in range(KT):
                    nc.tensor.matmul(qps_a[:, h * 256:(h + 1) * 256],
                                     lhsT=wq_t[t][:, h * 128:(h + 1) * 128],
                                     rhs=hs_t[t][:, s0:s0 + 256],
                                     start=(t == 0), stop=(t == KT - 1))
            for h in range(2):
                for t in range(KT):
                    nc.tensor.matmul(qps_b[:, h * 256:(h + 1) * 256],
                                     lhsT=wq_t[t][:, (h + 2) * 128:(h + 3) * 128],
                                     rhs=hs_t[t][:, s0:s0 + 256],
                                     start=(t == 0), stop=(t == KT - 1))
            for t in range(KT):
                nc.tensor.matmul(kv_ps[:, 0:256], lhsT=wk_t[t],
                                 rhs=hs_t[t][:, s0:s0 + 256],
                                 start=(t == 0), stop=(t == KT - 1))
            for j in range(2):
                for t in range(KT):
                    nc.tensor.matmul(kv_ps[:, 256 + j * 128:256 + (j + 1) * 128],
                                     lhsT=hs_t[t][:, s0 + j * 128:s0 + (j + 1) * 128],
                                     rhs=wv_t[t], start=(t == 0), stop=(t == KT - 1))
            for h in range(2):
                nc.vector.tensor_copy(qT[h][:, s0:s0 + 256],
                                      qps_a[:, h * 256:(h + 1) * 256])
                nc.scalar.copy(qT[h + 2][:, s0:s0 + 256],
                               qps_b[:, h * 256:(h + 1) * 256])
            nc.vector.tensor_copy(kT[:, s0:s0 + 256], kv_ps[:, 0:256])
            # v blocks 2*hc, 2*hc+1 -> v[:, blk*128:(blk+1)*128]
            nc.vector.tensor_copy(v[:, s0:s0 + 256], kv_ps[:, 256:512])

        if debug == 1:
            # dump projections: out[0..3]=qT, out[4]=kT, out[5]=v
            for e in range(4):
                for h in range(GH):
                    nc.sync.dma_start(out=out[h, e], in_=qT[h][:, e * 512:(e + 1) * 512])
                nc.sync.dma_start(out=out[4, e], in_=kT[:, e * 512:(e + 1) * 512])
                nc.sync.dma_start(out=out[5, e], in_=v[:, e * 512:(e + 1) * 512])

        # ---- Phase 2+3: banded attention (qb-outer) + Wo row-tiles ----
        # per (h, qb): kjs = [max(0, qb-8) .. qb]; score blocks [128k x 128q]
        # accumulated transposed; exp batches of <=4 blocks per PSUM bank.
        pending = []   # (avden, pts, pt, kj_list, first, last, h, qb)

        def flush_one():
            # av accumulates alone as the bank's open group; den is issued as
            # one contiguous group into the same bank only after av has closed
            # (a start=True clears accumulation bits bank-wide).
            avden, pts, pt, kjl, first, last, h, qb = pending.pop(0)
            n = len(kjl)
            for i, kj in enumerate(kjl):
                nc.tensor.matmul(avden[:, 0:128], lhsT=v[:, kj * 128:(kj + 1) * 128],
                                 rhs=pt[:, i * 128:(i + 1) * 128],
                                 start=(first and i == 0), stop=(last and i == n - 1))
            if last:
                nkj = sum(len(bk) for _, bk in pts)
                d = 0
                for ptt, bk in pts:
                    for i in range(len(bk)):
                        nc.tensor.matmul(avden[:, 128:256], lhsT=ones,
                                         rhs=ptt[:, i * 128:(i + 1) * 128],
                                         start=(d == 0), stop=(d == nkj - 1))
                        d += 1
                rcb = smalls.tile([128, 128], dt.float32, tag="rcb")
                with nc.allow_low_precision(reason="fp32 reciprocal, full precision"):
                    nc.vector.reciprocal(rcb, avden[:, 128:256])
                nc.vector.tensor_mul(ohT[h][:, qb * 128:(qb + 1) * 128],
                                     avden[:, 0:128], rcb)

        for qb in range(NB if debug != 1 else 0):
            for h in range(GH):
                kjs = list(range(max(0, qb - 8), qb + 1))
                avden = psum.tile([128, 512], dt.float32, tag="ps", name=f"ad{qb}_{h}")
                qs = qT[h][:, qb * 128:(qb + 1) * 128]
                pts = []
                for bi in range(0, len(kjs), 4):
                    bk = kjs[bi:bi + 4]
                    sps = psum.tile([128, 512], dt.float32, tag="ps")
                    for i, kj in enumerate(bk):
                        mi = 0 if kj == qb else (1 if kj == qb - 8 else None)
                        if mi is not None:
                            nc.tensor.matmul(sps[:, i * 128:(i + 1) * 128],
                                             lhsT=ident, rhs=mask_t[mi],
                                             start=True, stop=False)
                        nc.tensor.matmul(sps[:, i * 128:(i + 1) * 128],
                                         lhsT=kT[:, kj * 128:(kj + 1) * 128],
                                         rhs=qs, start=(mi is None), stop=True)
                    pt = ptp.tile([128, 512], dt.bfloat16, tag="pt")
                    nc.scalar.activation(pt[:, :128 * len(bk)], sps[:, :128 * len(bk)],
                                         mybir.ActivationFunctionType.Exp, scale=SCALE)
                    pts.append((pt, bk))
                    pending.append((avden, pts, pt, bk, bi == 0, bi + 4 >= len(kjs), h, qb))
                    while len(pending) > pipe_depth:
                        flush_one()
            # Wo row-tile st=qb-1 (lag one qb so divides have definitely issued)
            if qb >= 1:
                emit_wo(nc, wops_p, outp, ohT, wo_t, out, qb - 1)
        while pending:
            flush_one()
        if debug != 1:
            emit_wo(nc, wops_p, outp, ohT, wo_t, out, NB - 1)

    nc.compile()
    return nc


def emit_wo(nc, wops_p, outp, ohT, wo_t, out, st):
    osb = outp.tile([128, 4 * 512], dt.bfloat16, tag="osb")
    for e in range(4):
        wops = wops_p.tile([128, 512], dt.float32, tag="wo")
        for ct in range(4):
            nc.tensor.matmul(wops, lhsT=ohT[ct][:, st * 128:(st + 1) * 128],
                             rhs=wo_t[ct][:, e * 512:(e + 1) * 512],
                             start=(ct == 0), stop=(ct == 3))
        nc.vector.tensor_copy(osb[:, e * 512:(e + 1) * 512], wops)
    nc.sync.dma_start(out=out[st].rearrange("e p n -> p e n"), in_=osb)


def _build_masks():
    kk = np.arange(128)[:, None]
    qq = np.arange(128)[None, :]
    diag = np.where(kk <= qq, 0.0, NEG).astype(np.float32)   # causal within diag block
    edge = np.where(kk >= qq, 0.0, NEG).astype(np.float32)   # window lower edge
    return np.stack([diag, edge])


def kernel(hidden_states, Wq, Wk, Wv, Wo):
    global _nc_cache
    if _nc_cache is None:
        _nc_cache = _build_nc()
    nc = _nc_cache

    bf16 = dt.np(dt.bfloat16)
    masks = _build_masks().astype(bf16)
    hsT = []
    for b in range(B):
        ht = np.ascontiguousarray(hidden_states[b].T.astype(bf16))    # [H, S]
        hsT.append(np.ascontiguousarray(ht.reshape(KT, 128, S)))
    in_maps = []
    for b in range(B):
        for gi in range(KV_HEADS):
            in_maps.append({
                "hsT": hsT[b],
                "wq": np.ascontiguousarray(Wq[:, gi * GD:(gi + 1) * GD].astype(bf16)),
                "wk": np.ascontiguousarray(Wk[:, gi * D:(gi + 1) * D].astype(bf16)),
                "wv": np.ascontiguousarray(Wv[:, gi * D:(gi + 1) * D].astype(bf16)),
                "wo": np.ascontiguousarray(Wo[gi * GD:(gi + 1) * GD, :].astype(bf16)),
                "masks": masks,
            })
    res = run_bass_kernel_spmd(nc, in_maps, list(range(8)))
    out = np.zeros((B, S, H), np.float32)
    for b in range(B):
        acc = None
        for gi in range(KV_HEADS):
            o = res.results[b * KV_HEADS + gi]["out"].astype(np.float32)
            acc = o if acc is None else acc + o
        out[b] = acc.transpose(0, 2, 1, 3).reshape(S, H)              # [16,4,128,512] -> [S,H]
    return out


# revision 32
# speedup vs baseline: 1.9404x; 1.0384x over previous
"""GQA with sliding-window + ALiBi (reduces to banded causal attention) on 8 TRN2 cores.

Sharding: 8 cores = 2 batches x 4 kv-head groups. Each core computes, for its
(batch b, kv group gi): Q projection for its 4 query heads, K/V projection for
its 1 kv head, banded sliding-window attention (window 1024, causal), and a
partial row-parallel Wo matmul. Host sums the 4 partials per batch.

Math notes (exact reductions of the reference):
- ALiBi bias is -clip(j-i,0)*slope: zero on all causal positions, nonzero only
  where the causal mask kills the score -> drop it entirely.
- The sliding mask adds +1.0 uniformly inside the window: softmax-invariant.
- Out-of-window/causal positions get -1e9 -> exp underflows to exactly 0.
- Scores are O(1), so softmax without max-subtraction is safe.

Implementation (v2):
- All matmul operands and DMA traffic in bf16 (PSUM accumulation stays fp32);
  rel-err gate is 2e-2, bf16 keeps us ~1e-2 or better.
- Whole hsT slab persists in SBUF (64KB/partition bf16), loaded once.
- V is produced pre-transposed ([s,d] layout) straight from the projection.
- 128-wide q blocks: only two mask patterns (causal diag, window edge),
  preloaded into PSUM by a PE matmul so score matmuls accumulate onto them.
- softmax denominator: ones[128,128]@pt accumulated alongside av in the same
  PSUM bank; one DVE divide produces oh = av/den.
- Attention is software-pipelined (scores -> exp -> av/den lag queue) and
  iterated qb-outer so each Wo row-tile issues as soon as its q block is done.
"""
import math
from contextlib import ExitStack

import numpy as np

import concourse.tile as tile
from concourse import bacc, mybir
from concourse.bass_utils import run_bass_kernel_spmd
from concourse.masks import make_identity

dt = mybir.dt

B, S, H = 2, 2048, 2048
NUM_HEADS, KV_HEADS, D = 16, 4, 128
WINDOW = 1024
GH = 4            # query heads per kv head (per core)
GD = GH * D       # 512: per-core slice of the hidden dim
SCALE = 1.0 / math.sqrt(D)
NEG = -1e9
NB = S // 128     # 16 128-wide blocks along s
KT = H // 128     # 16 contraction tiles for projections

_nc_cache = None


def _build_nc(pipe_depth=3, main_bufs=6, wo_bufs=2, pt_bufs=8, osb_bufs=4, debug=0,
              mask_via_act=False, pe_warmup=1):
    nc = bacc.Bacc()
    hsT = nc.declare_dram_parameter("hsT", [KT, 128, S], dt.bfloat16, isOutput=False)
    wq = nc.declare_dram_parameter("wq", [H, GD], dt.bfloat16, isOutput=False)
    wk = nc.declare_dram_parameter("wk", [H, D], dt.bfloat16, isOutput=False)
    wv = nc.declare_dram_parameter("wv", [H, D], dt.bfloat16, isOutput=False)
    wo = nc.declare_dram_parameter("wo", [GD, H], dt.bfloat16, isOutput=False)
    masks = nc.declare_dram_parameter("masks", [2, 128, 128], dt.bfloat16, isOutput=False)
    out = nc.declare_dram_parameter("out", [NB, 4, 128, 512], dt.bfloat16, isOutput=True)

    with tile.TileContext(nc) as tc, ExitStack() as ctx:
        consts = ctx.enter_context(tc.tile_pool(name="consts", bufs=1))
        wpool = ctx.enter_context(tc.tile_pool(name="wpool", bufs=1))
        big = ctx.enter_context(tc.tile_pool(name="big", bufs=1))
        ptp = ctx.enter_context(tc.tile_pool(name="ptp", bufs=pt_bufs))
        smalls = ctx.enter_context(tc.tile_pool(name="smalls", bufs=4))
        outp = ctx.enter_context(tc.tile_pool(name="outp", bufs=osb_bufs))
        psum = ctx.enter_context(tc.tile_pool(name="psum", bufs=main_bufs, space="PSUM"))
        wops_p = ctx.enter_context(tc.tile_pool(name="wops", bufs=wo_bufs, space="PSUM"))

        # constants
        ident32 = consts.tile([128, 128], dt.float32)
        make_identity(nc, ident32)
        ident = consts.tile([128, 128], dt.bfloat16)
        nc.vector.tensor_copy(ident, ident32)
        ones32 = consts.tile([128, 128], dt.float32)
        nc.vector.memset(ones32, 1.0)
        ones = consts.tile([128, 128], dt.bfloat16)
        nc.vector.tensor_copy(ones, ones32)
        # PE warm-up: keep the PE busy through the initial DMA window so the
        # p-state ramp completes before real matmuls, and preload the Exp
        # activation table so the first attention batch doesn't pay it.
        warm_ps = psum.tile([128, 512], dt.float32, tag="ps", name="warm")
        for i in range(pe_warmup):
            nc.tensor.matmul(warm_ps[:, 0:128], lhsT=ident, rhs=ones,
                             start=True, stop=True)
        warm_sb = smalls.tile([128, 128], dt.float32, tag="warm_sb")
        nc.scalar.activation(warm_sb, warm_ps[:, 0:128],
                             mybir.ActivationFunctionType.Exp, scale=0.0)

        # weights + the whole hsT slab persist in SBUF (all bf16).
        # Few BIG multi-tile DMAs (HWDGE queue overhead is per-instruction).
        wq_all = wpool.tile([128, KT * GD], dt.bfloat16, tag="wq", name="wq_all")
        wk_all = wpool.tile([128, KT * D], dt.bfloat16, tag="wk", name="wk_all")
        wv_all = wpool.tile([128, KT * D], dt.bfloat16, tag="wv", name="wv_all")
        hs_all = wpool.tile([128, KT * S], dt.bfloat16, tag="hs", name="hs_all")
        wq_t = [wq_all[:, t * GD:(t + 1) * GD] for t in range(KT)]
        wk_t = [wk_all[:, t * D:(t + 1) * D] for t in range(KT)]
        wv_t = [wv_all[:, t * D:(t + 1) * D] for t in range(KT)]
        hs_t = [hs_all[:, t * S:(t + 1) * S] for t in range(KT)]
        mask_t = []
        for i in range(2):
            mt = consts.tile([128, 128], dt.bfloat16, tag=f"mask{i}", name=f"mask{i}")
            mask_t.append(mt)
        wo_all = wpool.tile([128, 4 * H], dt.bfloat16, tag="wo", name="wo_all")
        wo_t = [wo_all[:, ct * H:(ct + 1) * H] for ct in range(4)]

        # persistent activations
        qT = [big.tile([128, S], dt.bfloat16, tag=f"qT{h}", name=f"qT{h}") for h in range(GH)]
        kT = big.tile([128, S], dt.bfloat16, tag="kT")
        v = big.tile([128, S], dt.bfloat16, tag="v")  # [s%128, (sblk, d)]
        ohT = [big.tile([128, S], dt.bfloat16, tag=f"ohT{h}", name=f"ohT{h}") for h in range(GH)]

        # ---- DMA issue: first chunk's operands first, then the rest ----
        # wq as two halves (t 0-7, 8-15) so the first q-group starts sooner;
        # hs chunk DMAs carry all 16 t-tiles' 512-column slices in one instr.
        # SBUF-side DMA APs need the partition dim outermost; DRAM side is
        # rearranged to the same p-outer element order.
        def dma_hs_chunk(ch, t0, t1):
            nc.sync.dma_start(
                out=hs_all.rearrange("p (t s) -> p t s", t=KT)[:, t0:t1, ch * 512:(ch + 1) * 512],
                in_=hsT.rearrange("t p s -> p t s")[:, t0:t1, ch * 512:(ch + 1) * 512])

        wq_sb = wq_all.rearrange("p (t n) -> p t n", t=KT)
        wq_dr = wq.rearrange("(t p) n -> p t n", t=KT)
        for t0 in range(0, 4, 2):
            nc.sync.dma_start(out=wq_sb[:, t0:t0 + 2], in_=wq_dr[:, t0:t0 + 2])
            dma_hs_chunk(0, t0, t0 + 2)
        for t0 in range(4, KT, 4):
            nc.sync.dma_start(out=wq_sb[:, t0:t0 + 4], in_=wq_dr[:, t0:t0 + 4])
            dma_hs_chunk(0, t0, t0 + 4)
        nc.sync.dma_start(out=wk_all.rearrange("p (t n) -> p t n", t=KT),
                          in_=wk.rearrange("(t p) n -> p t n", t=KT))
        nc.sync.dma_start(out=wv_all.rearrange("p (t n) -> p t n", t=KT),
                          in_=wv.rearrange("(t p) n -> p t n", t=KT))
        for ch in range(1, 4):
            dma_hs_chunk(ch, 0, 16)
        for i in range(2):
            nc.sync.dma_start(out=mask_t[i], in_=masks[i])
        nc.sync.dma_start(out=wo_all.rearrange("p (t n) -> p t n", t=4),
                          in_=wo.rearrange("(t p) n -> p t n", t=4))

        # ---- Phase 1: projections per 256-wide half-chunk (3 PSUM banks each) ----
        # bank qps2: [h0 | h1] halves; qps2b: [h2 | h3]; bank kv: [k | v0 | v1]
        # NOTE: matmul start=True clears accumulation state for the WHOLE PSUM
        # bank, so co-resident groups in one bank must be issued contiguously
        # (a group fully closes before the next group's start): t-inner loops.
        # Chunk 0 runs t-major with one bank per q head so the PE can consume
        # quarter-granularity DMA arrivals without inter-group hazards.
        q_ps0 = [psum.tile([128, 512], dt.float32, tag="ps", name=f"q0_{h}")
                 for h in range(GH)]
        for t in range(KT):
            for h in range(GH):
                nc.tensor.matmul(q_ps0[h], lhsT=wq_t[t][:, h * 128:(h + 1) * 128],
                                 rhs=hs_t[t][:, 0:512],
                                 start=(t == 0), stop=(t == KT - 1))
        kv0_k = psum.tile([128, 512], dt.float32, tag="ps", name="k0")
        for t in range(KT):
            nc.tensor.matmul(kv0_k, lhsT=wk_t[t], rhs=hs_t[t][:, 0:512],
                             start=(t == 0), stop=(t == KT - 1))
        kv0_v = psum.tile([128, 512], dt.float32, tag="ps", name="v0")
        for j in range(4):
            for t in range(KT):
                nc.tensor.matmul(kv0_v[:, j * 128:(j + 1) * 128],
                                 lhsT=hs_t[t][:, j * 128:(j + 1) * 128],
                                 rhs=wv_t[t], start=(t == 0), stop=(t == KT - 1))
        for h in range(2):
            nc.vector.tensor_copy(qT[h][:, 0:512], q_ps0[h])
            nc.scalar.copy(qT[h + 2][:, 0:512], q_ps0[h + 2])
        nc.vector.tensor_copy(kT[:, 0:512], kv0_k)
        nc.vector.tensor_copy(v[:, 0:512], kv0_v)

        for hc in range(2, 8):
            s0 = hc * 256
            qps_a = psum.tile([128, 512], dt.float32, tag="ps", name=f"qa{hc}")
            qps_b = psum.tile([128, 512], dt.float32, tag="ps", name=f"qb{hc}")
            kv_ps = psum.tile([128, 512], dt.float32, tag="ps", name=f"kv{hc}")
            for h in range(2):
                for t in range(KT):
                    nc.tensor.matmul(qps_a[:, h * 256:(h + 1) * 256],
                                     lhsT=wq_t[t][:, h * 128:(h + 1) * 128],
                                     rhs=hs_t[t][:, s0:s0 + 256],
                                     start=(t == 0), stop=(t == KT - 1))
            for h in range(2):
                for t in range(KT):
                    nc.tensor.matmul(qps_b[:, h * 256:(h + 1) * 256],
                                     lhsT=wq_t[t][:, (h + 2) * 128:(h + 3) * 128],
                                     rhs=hs_t[t][:, s0:s0 + 256],
                                     start=(t == 0), stop=(t == KT - 1))
            for t in range(KT):
                nc.tensor.matmul(kv_ps[:, 0:256], lhsT=wk_t[t],
                                 rhs=hs_t[t][:, s0:s0 + 256],
                                 start=(t == 0), stop=(t == KT - 1))
            for j in range(2):
                for t in range(KT):
                    nc.tensor.matmul(kv_ps[:, 256 + j * 128:256 + (j + 1) * 128],
                                     lhsT=hs_t[t][:, s0 + j * 128:s0 + (j + 1) * 128],
                                     rhs=wv_t[t], start=(t == 0), stop=(t == KT - 1))
            for h in range(2):
                nc.vector.tensor_copy(qT[h][:, s0:s0 + 256],
                                      qps_a[:, h * 256:(h + 1) * 256])
                nc.scalar.copy(qT[h + 2][:, s0:s0 + 256],
                               qps_b[:, h * 256:(h + 1) * 256])
            nc.vector.tensor_copy(kT[:, s0:s0 + 256], kv_ps[:, 0:256])
            # v blocks 2*hc, 2*hc+1 -> v[:, blk*128:(blk+1)*128]
            nc.vector.tensor_copy(v[:, s0:s0 + 256], kv_ps[:, 256:512])

        if debug == 1:
            # dump projections: out[0..3]=qT, out[4]=kT, out[5]=v
            for e in range(4):
                for h in range(GH):
                    nc.sync.dma_start(out=out[h, e], in_=qT[h][:, e * 512:(e + 1) * 512])
                nc.sync.dma_start(out=out[4, e], in_=kT[:, e * 512:(e + 1) * 512])
                nc.sync.dma_start(out=out[5, e], in_=v[:, e * 512:(e + 1) * 512])

        # ---- Phase 2+3: banded attention (qb-outer) + Wo row-tiles ----
        # per (h, qb): kjs = [max(0, qb-8) .. qb]; score blocks [128k x 128q]
        # accumulated transposed; exp batches of <=4 blocks per PSUM bank.
        pending = []   # (avden, pts, pt, kj_list, first, last, h, qb)

        def flush_one():
            # av accumulates alone as the bank's open group; den is issued as
            # one contiguous group into the same bank only after av has closed
            # (a start=True clears accumulation bits bank-wide).
            avden, pts, pt, kjl, first, last, h, qb = pending.pop(0)
            n = len(kjl)
            for i, kj in enumerate(kjl):
                nc.tensor.matmul(avden[:, 0:128], lhsT=v[:, kj * 128:(kj + 1) * 128],
                                 rhs=pt[:, i * 128:(i + 1) * 128],
                                 start=(first and i == 0), stop=(last and i == n - 1))
            if last:
                nkj = sum(len(bk) for _, bk in pts)
                d = 0
                for ptt, bk in pts:
                    for i in range(len(bk)):
                        nc.tensor.matmul(avden[:, 128:256], lhsT=ones,
                                         rhs=ptt[:, i * 128:(i + 1) * 128],
                                         start=(d == 0), stop=(d == nkj - 1))
                        d += 1
                rcb = smalls.tile([128, 128], dt.float32, tag="rcb")
                with nc.allow_low_precision(reason="fp32 reciprocal, full precision"):
                    nc.vector.reciprocal(rcb, avden[:, 128:256])
                nc.vector.tensor_mul(ohT[h][:, qb * 128:(qb + 1) * 128],
                                     avden[:, 0:128], rcb)

        for qb in range(NB if debug != 1 else 0):
            for h in range(GH):
                kjs = list(range(max(0, qb - 8), qb + 1))
                avden = psum.tile([128, 512], dt.float32, tag="ps", name=f"ad{qb}_{h}")
                qs = qT[h][:, qb * 128:(qb + 1) * 128]
                pts = []
                for bi in range(0, len(kjs), 4):
                    bk = kjs[bi:bi + 4]
                    sps = psum.tile([128, 512], dt.float32, tag="ps")
                    for i, kj in enumerate(bk):
                        mi = 0 if kj == qb else (1 if kj == qb - 8 else None)
                        if mi is not None:
                            if mask_via_act:
                                nc.scalar.copy(sps[:, i * 128:(i + 1) * 128],
                                               mask_t[mi])
                            else:
                                nc.tensor.matmul(sps[:, i * 128:(i + 1) * 128],
                                                 lhsT=ident, rhs=mask_t[mi],
                                                 start=True, stop=False)
                        nc.tensor.matmul(sps[:, i * 128:(i + 1) * 128],
                                         lhsT=kT[:, kj * 128:(kj + 1) * 128],
                                         rhs=qs, start=(mi is None and not mask_via_act),
                                         stop=True)
                    pt = ptp.tile([128, 512], dt.bfloat16, tag="pt")
                    nc.scalar.activation(pt[:, :128 * len(bk)], sps[:, :128 * len(bk)],
                                         mybir.ActivationFunctionType.Exp, scale=SCALE)
                    pts.append((pt, bk))
                    pending.append((avden, pts, pt, bk, bi == 0, bi + 4 >= len(kjs), h, qb))
                    while len(pending) > pipe_depth:
                        flush_one()
            # Wo row-tile st=qb-1 (lag one qb so divides have definitely issued)
            if qb >= 1:
                emit_wo(nc, wops_p, outp, ohT, wo_t, out, qb - 1)
        while pending:
            flush_one()
        if debug != 1:
            emit_wo(nc, wops_p, outp, ohT, wo_t, out, NB - 1, split_dma=True)

    nc.compile()
    return nc


def emit_wo(nc, wops_p, outp, ohT, wo_t, out, st, split_dma=False):
    osb = outp.tile([128, 4 * 512], dt.bfloat16, tag="osb")
    for e in range(4):
        wops = wops_p.tile([128, 512], dt.float32, tag="wo")
        for ct in range(4):
            nc.tensor.matmul(wops, lhsT=ohT[ct][:, st * 128:(st + 1) * 128],
                             rhs=wo_t[ct][:, e * 512:(e + 1) * 512],
                             start=(ct == 0), stop=(ct == 3))
        nc.vector.tensor_copy(osb[:, e * 512:(e + 1) * 512], wops)
        if split_dma:
            nc.sync.dma_start(out=out[st, e], in_=osb[:, e * 512:(e + 1) * 512])
    if not split_dma:
        nc.sync.dma_start(out=out[st].rearrange("e p n -> p e n"), in_=osb)


def _build_masks():
    kk = np.arange(128)[:, None]
    qq = np.arange(128)[None, :]
    diag = np.where(kk <= qq, 0.0, NEG).astype(np.float32)   # causal within diag block
    edge = np.where(kk >= qq, 0.0, NEG).astype(np.float32)   # window lower edge
    return np.stack([diag, edge])


def kernel(hidden_states, Wq, Wk, Wv, Wo):
    global _nc_cache
    if _nc_cache is None:
        _nc_cache = _build_nc()
    nc = _nc_cache

    bf16 = dt.np(dt.bfloat16)
    masks = _build_masks().astype(bf16)
    hsT = []
    for b in range(B):
        ht = np.ascontiguousarray(hidden_states[b].T.astype(bf16))    # [H, S]
        hsT.append(np.ascontiguousarray(ht.reshape(KT, 128, S)))
    in_maps = []
    for b in range(B):
        for gi in range(KV_HEADS):
            in_maps.append({
                "hsT": hsT[b],
                "wq": np.ascontiguousarray(Wq[:, gi * GD:(gi + 1) * GD].astype(bf16)),
                "wk": np.ascontiguousarray(Wk[:, gi * D:(gi + 1) * D].astype(bf16)),
                "wv": np.ascontiguousarray(Wv[:, gi * D:(gi + 1) * D].astype(bf16)),
                "wo": np.ascontiguousarray(Wo[gi * GD:(gi + 1) * GD, :].astype(bf16)),
                "masks": masks,
            })
    res = run_bass_kernel_spmd(nc, in_maps, list(range(8)))
    out = np.zeros((B, S, H), np.float32)
    for b in range(B):
        acc = None
        for gi in range(KV_HEADS):
            o = res.results[b * KV_HEADS + gi]["out"].astype(np.float32)
            acc = o if acc is None else acc + o
        out[b] = acc.transpose(0, 2, 1, 3).reshape(S, H)              # [16,4,128,512] -> [S,H]
    return out


# revision 34
# speedup vs baseline: 1.9407x; 1.0002x over previous
"""GQA with sliding-window + ALiBi (reduces to banded causal attention) on 8 TRN2 cores.

Sharding: 8 cores = 2 batches x 4 kv-head groups. Each core computes, for its
(batch b, kv group gi): Q projection for its 4 query heads, K/V projection for
its 1 kv head, banded sliding-window attention (window 1024, causal), and a
partial row-parallel Wo matmul. Host sums the 4 partials per batch.

Math notes (exact reductions of the reference):
- ALiBi bias is -clip(j-i,0)*slope: zero on all causal positions, nonzero only
  where the causal mask kills the score -> drop it entirely.
- The sliding mask adds +1.0 uniformly inside the window: softmax-invariant.
- Out-of-window/causal positions get -1e9 -> exp underflows to exactly 0.
- Scores are O(1), so softmax without max-subtraction is safe.

Implementation (v2):
- All matmul operands and DMA traffic in bf16 (PSUM accumulation stays fp32);
  rel-err gate is 2e-2, bf16 keeps us ~1e-2 or better.
- Whole hsT slab persists in SBUF (64KB/partition bf16), loaded once.
- V is produced pre-transposed ([s,d] layout) straight from the projection.
- 128-wide q blocks: only two mask patterns (causal diag, window edge),
  preloaded into PSUM by a PE matmul so score matmuls accumulate onto them.
- softmax denominator: ones[128,128]@pt accumulated alongside av in the same
  PSUM bank; one DVE divide produces oh = av/den.
- Attention is software-pipelined (scores -> exp -> av/den lag queue) and
  iterated qb-outer so each Wo row-tile issues as soon as its q block is done.
"""
import math
from contextlib import ExitStack

import numpy as np

import concourse.tile as tile
from concourse import bacc, mybir
from concourse.bass_utils import run_bass_kernel_spmd
from concourse.masks import make_identity

dt = mybir.dt

B, S, H = 2, 2048, 2048
NUM_HEADS, KV_HEADS, D = 16, 4, 128
WINDOW = 1024
GH = 4            # query heads per kv head (per core)
GD = GH * D       # 512: per-core slice of the hidden dim
SCALE = 1.0 / math.sqrt(D)
NEG = -1e9
NB = S // 128     # 16 128-wide blocks along s
KT = H // 128     # 16 contraction tiles for projections

_nc_cache = None


def _build_nc(pipe_depth=3, main_bufs=6, wo_bufs=2, pt_bufs=8, osb_bufs=4, debug=0,
              mask_via_act=False, pe_warmup=1):
    nc = bacc.Bacc()
    hsT = nc.declare_dram_parameter("hsT", [KT, 128, S], dt.bfloat16, isOutput=False)
    wq = nc.declare_dram_parameter("wq", [H, GD], dt.bfloat16, isOutput=False)
    wk = nc.declare_dram_parameter("wk", [H, D], dt.bfloat16, isOutput=False)
    wv = nc.declare_dram_parameter("wv", [H, D], dt.bfloat16, isOutput=False)
    wo = nc.declare_dram_parameter("wo", [GD, H], dt.bfloat16, isOutput=False)
    masks = nc.declare_dram_parameter("masks", [2, 128, 128], dt.bfloat16, isOutput=False)
    out = nc.declare_dram_parameter("out", [NB, 4, 128, 512], dt.bfloat16, isOutput=True)

    with tile.TileContext(nc) as tc, ExitStack() as ctx:
        consts = ctx.enter_context(tc.tile_pool(name="consts", bufs=1))
        wpool = ctx.enter_context(tc.tile_pool(name="wpool", bufs=1))
        big = ctx.enter_context(tc.tile_pool(name="big", bufs=1))
        ptp = ctx.enter_context(tc.tile_pool(name="ptp", bufs=pt_bufs))
        smalls = ctx.enter_context(tc.tile_pool(name="smalls", bufs=4))
        outp = ctx.enter_context(tc.tile_pool(name="outp", bufs=osb_bufs))
        psum = ctx.enter_context(tc.tile_pool(name="psum", bufs=main_bufs, space="PSUM"))
        wops_p = ctx.enter_context(tc.tile_pool(name="wops", bufs=wo_bufs, space="PSUM"))

        # constants
        ident32 = consts.tile([128, 128], dt.float32)
        make_identity(nc, ident32)
        ident = consts.tile([128, 128], dt.bfloat16)
        nc.vector.tensor_copy(ident, ident32)
        ones32 = consts.tile([128, 128], dt.float32)
        nc.vector.memset(ones32, 1.0)
        ones = consts.tile([128, 128], dt.bfloat16)
        nc.vector.tensor_copy(ones, ones32)
        # PE warm-up: keep the PE busy through the initial DMA window so the
        # p-state ramp completes before real matmuls, and preload the Exp
        # activation table so the first attention batch doesn't pay it.
        warm_ps = psum.tile([128, 512], dt.float32, tag="ps", name="warm")
        for i in range(pe_warmup):
            nc.tensor.matmul(warm_ps[:, 0:128], lhsT=ident, rhs=ones,
                             start=True, stop=True)
        warm_sb = smalls.tile([128, 128], dt.float32, tag="warm_sb")
        nc.scalar.activation(warm_sb, warm_ps[:, 0:128],
                             mybir.ActivationFunctionType.Exp, scale=0.0)

        # weights + the whole hsT slab persist in SBUF (all bf16).
        # Few BIG multi-tile DMAs (HWDGE queue overhead is per-instruction).
        wq_all = wpool.tile([128, KT * GD], dt.bfloat16, tag="wq", name="wq_all")
        wk_all = wpool.tile([128, KT * D], dt.bfloat16, tag="wk", name="wk_all")
        wv_all = wpool.tile([128, KT * D], dt.bfloat16, tag="wv", name="wv_all")
        hs_all = wpool.tile([128, KT * S], dt.bfloat16, tag="hs", name="hs_all")
        wq_t = [wq_all[:, t * GD:(t + 1) * GD] for t in range(KT)]
        wk_t = [wk_all[:, t * D:(t + 1) * D] for t in range(KT)]
        wv_t = [wv_all[:, t * D:(t + 1) * D] for t in range(KT)]
        hs_t = [hs_all[:, t * S:(t + 1) * S] for t in range(KT)]
        mask_t = []
        for i in range(2):
            mt = consts.tile([128, 128], dt.bfloat16, tag=f"mask{i}", name=f"mask{i}")
            mask_t.append(mt)
        wo_all = wpool.tile([128, 4 * H], dt.bfloat16, tag="wo", name="wo_all")
        wo_t = [wo_all[:, ct * H:(ct + 1) * H] for ct in range(4)]

        # persistent activations
        qT = [big.tile([128, S], dt.bfloat16, tag=f"qT{h}", name=f"qT{h}") for h in range(GH)]
        kT = big.tile([128, S], dt.bfloat16, tag="kT")
        v = big.tile([128, S], dt.bfloat16, tag="v")  # [s%128, (sblk, d)]
        ohT = [big.tile([128, S], dt.bfloat16, tag=f"ohT{h}", name=f"ohT{h}") for h in range(GH)]

        # ---- DMA issue: first chunk's operands first, then the rest ----
        # wq as two halves (t 0-7, 8-15) so the first q-group starts sooner;
        # hs chunk DMAs carry all 16 t-tiles' 512-column slices in one instr.
        # SBUF-side DMA APs need the partition dim outermost; DRAM side is
        # rearranged to the same p-outer element order.
        def dma_hs_chunk(ch, t0, t1):
            nc.sync.dma_start(
                out=hs_all.rearrange("p (t s) -> p t s", t=KT)[:, t0:t1, ch * 512:(ch + 1) * 512],
                in_=hsT.rearrange("t p s -> p t s")[:, t0:t1, ch * 512:(ch + 1) * 512])

        wq_sb = wq_all.rearrange("p (t n) -> p t n", t=KT)
        wq_dr = wq.rearrange("(t p) n -> p t n", t=KT)
        for t0 in range(0, 4, 2):
            nc.sync.dma_start(out=wq_sb[:, t0:t0 + 2], in_=wq_dr[:, t0:t0 + 2])
            dma_hs_chunk(0, t0, t0 + 2)
        for t0 in range(4, KT, 4):
            nc.sync.dma_start(out=wq_sb[:, t0:t0 + 4], in_=wq_dr[:, t0:t0 + 4])
            dma_hs_chunk(0, t0, t0 + 4)
        nc.sync.dma_start(out=wk_all.rearrange("p (t n) -> p t n", t=KT),
                          in_=wk.rearrange("(t p) n -> p t n", t=KT))
        nc.sync.dma_start(out=wv_all.rearrange("p (t n) -> p t n", t=KT),
                          in_=wv.rearrange("(t p) n -> p t n", t=KT))
        for ch in range(1, 4):
            dma_hs_chunk(ch, 0, 16)
        for i in range(2):
            nc.sync.dma_start(out=mask_t[i], in_=masks[i])
        nc.sync.dma_start(out=wo_all.rearrange("p (t n) -> p t n", t=4),
                          in_=wo.rearrange("(t p) n -> p t n", t=4))

        # ---- Phase 1: projections per 256-wide half-chunk (3 PSUM banks each) ----
        # bank qps2: [h0 | h1] halves; qps2b: [h2 | h3]; bank kv: [k | v0 | v1]
        # NOTE: matmul start=True clears accumulation state for the WHOLE PSUM
        # bank, so co-resident groups in one bank must be issued contiguously
        # (a group fully closes before the next group's start): t-inner loops.
        # Chunk 0 runs t-major with one bank per q head so the PE can consume
        # quarter-granularity DMA arrivals without inter-group hazards.
        q_ps0 = [psum.tile([128, 512], dt.float32, tag="ps", name=f"q0_{h}")
                 for h in range(GH)]
        for t in range(KT):
            for h in range(GH):
                nc.tensor.matmul(q_ps0[h], lhsT=wq_t[t][:, h * 128:(h + 1) * 128],
                                 rhs=hs_t[t][:, 0:512],
                                 start=(t == 0), stop=(t == KT - 1))
        kv0_k = psum.tile([128, 512], dt.float32, tag="ps", name="k0")
        for t in range(KT):
            nc.tensor.matmul(kv0_k, lhsT=wk_t[t], rhs=hs_t[t][:, 0:512],
                             start=(t == 0), stop=(t == KT - 1))
        kv0_v = psum.tile([128, 512], dt.float32, tag="ps", name="v0")
        for j in range(4):
            for t in range(KT):
                nc.tensor.matmul(kv0_v[:, j * 128:(j + 1) * 128],
                                 lhsT=hs_t[t][:, j * 128:(j + 1) * 128],
                                 rhs=wv_t[t], start=(t == 0), stop=(t == KT - 1))
        for h in range(2):
            nc.vector.tensor_copy(qT[h][:, 0:512], q_ps0[h])
            nc.scalar.copy(qT[h + 2][:, 0:512], q_ps0[h + 2])
        nc.vector.tensor_copy(kT[:, 0:512], kv0_k)
        nc.vector.tensor_copy(v[:, 0:512], kv0_v)

        for hc in range(2, 8):
            s0 = hc * 256
            qps_a = psum.tile([128, 512], dt.float32, tag="ps", name=f"qa{hc}")
            qps_b = psum.tile([128, 512], dt.float32, tag="ps", name=f"qb{hc}")
            kv_ps = psum.tile([128, 512], dt.float32, tag="ps", name=f"kv{hc}")
            for h in range(2):
                for t in range(KT):
                    nc.tensor.matmul(qps_a[:, h * 256:(h + 1) * 256],
                                     lhsT=wq_t[t][:, h * 128:(h + 1) * 128],
                                     rhs=hs_t[t][:, s0:s0 + 256],
                                     start=(t == 0), stop=(t == KT - 1))
            for h in range(2):
                for t in range(KT):
                    nc.tensor.matmul(qps_b[:, h * 256:(h + 1) * 256],
                                     lhsT=wq_t[t][:, (h + 2) * 128:(h + 3) * 128],
                                     rhs=hs_t[t][:, s0:s0 + 256],
                                     start=(t == 0), stop=(t == KT - 1))
            for t in range(KT):
                nc.tensor.matmul(kv_ps[:, 0:256], lhsT=wk_t[t],
                                 rhs=hs_t[t][:, s0:s0 + 256],
                                 start=(t == 0), stop=(t == KT - 1))
            for j in range(2):
                for t in range(KT):
                    nc.tensor.matmul(kv_ps[:, 256 + j * 128:256 + (j + 1) * 128],
                                     lhsT=hs_t[t][:, s0 + j * 128:s0 + (j + 1) * 128],
                                     rhs=wv_t[t], start=(t == 0), stop=(t == KT - 1))
            for h in range(2):
                nc.vector.tensor_copy(qT[h][:, s0:s0 + 256],
                                      qps_a[:, h * 256:(h + 1) * 256])
                nc.scalar.copy(qT[h + 2][:, s0:s0 + 256],
                               qps_b[:, h * 256:(h + 1) * 256])
            nc.vector.tensor_copy(kT[:, s0:s0 + 256], kv_ps[:, 0:256])
            # v blocks 2*hc, 2*hc+1 -> v[:, blk*128:(blk+1)*128]
            nc.vector.tensor_copy(v[:, s0:s0 + 256], kv_ps[:, 256:512])

        if debug == 1:
            # dump projections: out[0..3]=qT, out[4]=kT, out[5]=v
            for e in range(4):
                for h in range(GH):
                    nc.sync.dma_start(out=out[h, e], in_=qT[h][:, e * 512:(e + 1) * 512])
                nc.sync.dma_start(out=out[4, e], in_=kT[:, e * 512:(e + 1) * 512])
                nc.sync.dma_start(out=out[5, e], in_=v[:, e * 512:(e + 1) * 512])

        # ---- Phase 2+3: banded attention (qb-outer) + Wo row-tiles ----
        # per (h, qb): kjs = [max(0, qb-8) .. qb]; score blocks [128k x 128q]
        # accumulated transposed; exp batches of <=4 blocks per PSUM bank.
        pending = []   # (avden, pts, pt, kj_list, first, last, h, qb)

        def flush_one():
            # av accumulates alone as the bank's open group; den is issued as
            # one contiguous group into the same bank only after av has closed
            # (a start=True clears accumulation bits bank-wide).
            avden, pts, pt, kjl, first, last, h, qb = pending.pop(0)
            n = len(kjl)
            for i, kj in enumerate(kjl):
                nc.tensor.matmul(avden[:, 0:128], lhsT=v[:, kj * 128:(kj + 1) * 128],
                                 rhs=pt[:, i * 128:(i + 1) * 128],
                                 start=(first and i == 0), stop=(last and i == n - 1))
            if last:
                nkj = sum(len(bk) for _, bk in pts)
                d = 0
                for ptt, bk in pts:
                    for i in range(len(bk)):
                        nc.tensor.matmul(avden[:, 128:256], lhsT=ones,
                                         rhs=ptt[:, i * 128:(i + 1) * 128],
                                         start=(d == 0), stop=(d == nkj - 1))
                        d += 1
                rcb = smalls.tile([128, 128], dt.float32, tag="rcb")
                with nc.allow_low_precision(reason="fp32 reciprocal, full precision"):
                    nc.vector.reciprocal(rcb, avden[:, 128:256])
                nc.vector.tensor_mul(ohT[h][:, qb * 128:(qb + 1) * 128],
                                     avden[:, 0:128], rcb)

        for qb in range(NB if debug != 1 else 0):
            for h in range(GH):
                kjs = list(range(max(0, qb - 8), qb + 1))
                avden = psum.tile([128, 512], dt.float32, tag="ps", name=f"ad{qb}_{h}")
                qs = qT[h][:, qb * 128:(qb + 1) * 128]
                pts = []
                for bi in range(0, len(kjs), 4):
                    bk = kjs[bi:bi + 4]
                    sps = psum.tile([128, 512], dt.float32, tag="ps")
                    for i, kj in enumerate(bk):
                        nc.tensor.matmul(sps[:, i * 128:(i + 1) * 128],
                                         lhsT=kT[:, kj * 128:(kj + 1) * 128],
                                         rhs=qs, start=True, stop=True)
                    pt = ptp.tile([128, 512], dt.bfloat16, tag="pt")
                    nc.scalar.activation(pt[:, :128 * len(bk)], sps[:, :128 * len(bk)],
                                         mybir.ActivationFunctionType.Exp, scale=SCALE)
                    # mask by zeroing exp weights (unmasked exp can't overflow:
                    # |score*scale| <= sqrt(128)*|q||k|*scale ~ O(12))
                    for i, kj in enumerate(bk):
                        mi = 0 if kj == qb else (1 if kj == qb - 8 else None)
                        if mi is not None:
                            nc.vector.tensor_mul(pt[:, i * 128:(i + 1) * 128],
                                                 pt[:, i * 128:(i + 1) * 128],
                                                 mask_t[mi])
                    pts.append((pt, bk))
                    pending.append((avden, pts, pt, bk, bi == 0, bi + 4 >= len(kjs), h, qb))
                    while len(pending) > pipe_depth:
                        flush_one()
            # Wo row-tile st=qb-1 (lag one qb so divides have definitely issued)
            if qb >= 1:
                emit_wo(nc, wops_p, outp, ohT, wo_t, out, qb - 1)
        while pending:
            flush_one()
        if debug != 1:
            emit_wo(nc, wops_p, outp, ohT, wo_t, out, NB - 1, split_dma=True)

    nc.compile()
    return nc


def emit_wo(nc, wops_p, outp, ohT, wo_t, out, st, split_dma=False):
    osb = outp.tile([128, 4 * 512], dt.bfloat16, tag="osb")
    for e in range(4):
        wops = wops_p.tile([128, 512], dt.float32, tag="wo")
        for ct in range(4):
            nc.tensor.matmul(wops, lhsT=ohT[ct][:, st * 128:(st + 1) * 128],
                             rhs=wo_t[ct][:, e * 512:(e + 1) * 512],
                             start=(ct == 0), stop=(ct == 3))
        nc.vector.tensor_copy(osb[:, e * 512:(e + 1) * 512], wops)
        if split_dma:
            nc.sync.dma_start(out=out[st, e], in_=osb[:, e * 512:(e + 1) * 512])
    if not split_dma:
        nc.sync.dma_start(out=out[st].rearrange("e p n -> p e n"), in_=osb)


def _build_masks():
    kk = np.arange(128)[:, None]
    qq = np.arange(128)[None, :]
    diag = (kk <= qq).astype(np.float32)   # causal within diag block
    edge = (kk >= qq).astype(np.float32)   # window lower edge
    return np.stack([diag, edge])


def kernel(hidden_states, Wq, Wk, Wv, Wo):
    global _nc_cache
    if _nc_cache is None:
        _nc_cache = _build_nc()
    nc = _nc_cache

    bf16 = dt.np(dt.bfloat16)
    masks = _build_masks().astype(bf16)
    hsT = []
    for b in range(B):
        ht = np.ascontiguousarray(hidden_states[b].T.astype(bf16))    # [H, S]
        hsT.append(np.ascontiguousarray(ht.reshape(KT, 128, S)))
    in_maps = []
    for b in range(B):
        for gi in range(KV_HEADS):
            in_maps.append({
                "hsT": hsT[b],
                "wq": np.ascontiguousarray(Wq[:, gi * GD:(gi + 1) * GD].astype(bf16)),
                "wk": np.ascontiguousarray(Wk[:, gi * D:(gi + 1) * D].astype(bf16)),
                "wv": np.ascontiguousarray(Wv[:, gi * D:(gi + 1) * D].astype(bf16)),
                "wo": np.ascontiguousarray(Wo[gi * GD:(gi + 1) * GD, :].astype(bf16)),
                "masks": masks,
            })
    res = run_bass_kernel_spmd(nc, in_maps, list(range(8)))
    out = np.zeros((B, S, H), np.float32)
    for b in range(B):
        acc = None
        for gi in range(KV_HEADS):
            o = res.results[b * KV_HEADS + gi]["out"].astype(np.float32)
            acc = o if acc is None else acc + o
        out[b] = acc.transpose(0, 2, 1, 3).reshape(S, H)              # [16,4,128,512] -> [S,H]
    return out


# revision 45
# speedup vs baseline: 1.9656x; 1.0128x over previous
"""GQA with sliding-window + ALiBi (reduces to banded causal attention) on 8 TRN2 cores.

Sharding: 8 cores = 2 batches x 4 kv-head groups. Each core computes, for its
(batch b, kv group gi): Q projection for its 4 query heads, K/V projection for
its 1 kv head, banded sliding-window attention (window 1024, causal), and a
partial row-parallel Wo matmul. Host sums the 4 partials per batch.

Math notes (exact reductions of the reference):
- ALiBi bias is -clip(j-i,0)*slope: zero on all causal positions, nonzero only
  where the causal mask kills the score -> drop it entirely.
- The sliding mask adds +1.0 uniformly inside the window: softmax-invariant.
- Out-of-window/causal positions get -1e9 -> exp underflows to exactly 0.
- Scores are O(1), so softmax without max-subtraction is safe.

Implementation (v2):
- All matmul operands and DMA traffic in bf16 (PSUM accumulation stays fp32);
  rel-err gate is 2e-2, bf16 keeps us ~1e-2 or better.
- Whole hsT slab persists in SBUF (64KB/partition bf16), loaded once.
- V is produced pre-transposed ([s,d] layout) straight from the projection.
- 128-wide q blocks: only two mask patterns (causal diag, window edge),
  preloaded into PSUM by a PE matmul so score matmuls accumulate onto them.
- softmax denominator: ones[128,128]@pt accumulated alongside av in the same
  PSUM bank; one DVE divide produces oh = av/den.
- Attention is software-pipelined (scores -> exp -> av/den lag queue) and
  iterated qb-outer so each Wo row-tile issues as soon as its q block is done.
"""
import math
from contextlib import ExitStack

import numpy as np

import concourse.tile as tile
from concourse import bacc, mybir
from concourse.bass_utils import run_bass_kernel_spmd
from concourse.masks import make_identity

dt = mybir.dt

B, S, H = 2, 2048, 2048
NUM_HEADS, KV_HEADS, D = 16, 4, 128
WINDOW = 1024
GH = 4            # query heads per kv head (per core)
GD = GH * D       # 512: per-core slice of the hidden dim
SCALE = 1.0 / math.sqrt(D)
NEG = -1e9
NB = S // 128     # 16 128-wide blocks along s
KT = H // 128     # 16 contraction tiles for projections

_nc_cache = None


def _build_nc(pipe_depth=5, main_bufs=6, wo_bufs=2, pt_bufs=10, osb_bufs=4, debug=0,
              mask_via_act=False, pe_warmup=1):
    nc = bacc.Bacc()
    hsT = nc.declare_dram_parameter("hsT", [KT, 128, S], dt.bfloat16, isOutput=False)
    wq = nc.declare_dram_parameter("wq", [H, GD], dt.bfloat16, isOutput=False)
    wk = nc.declare_dram_parameter("wk", [H, D], dt.bfloat16, isOutput=False)
    wv = nc.declare_dram_parameter("wv", [H, D], dt.bfloat16, isOutput=False)
    wo = nc.declare_dram_parameter("wo", [GD, H], dt.bfloat16, isOutput=False)
    masks = nc.declare_dram_parameter("masks", [2, 128, 128], dt.bfloat16, isOutput=False)
    out = nc.declare_dram_parameter("out", [NB, 4, 128, 512], dt.bfloat16, isOutput=True)

    with tile.TileContext(nc) as tc, ExitStack() as ctx:
        consts = ctx.enter_context(tc.tile_pool(name="consts", bufs=1))
        wpool = ctx.enter_context(tc.tile_pool(name="wpool", bufs=1))
        big = ctx.enter_context(tc.tile_pool(name="big", bufs=1))
        ptp = ctx.enter_context(tc.tile_pool(name="ptp", bufs=pt_bufs))
        smalls = ctx.enter_context(tc.tile_pool(name="smalls", bufs=4))
        outp = ctx.enter_context(tc.tile_pool(name="outp", bufs=osb_bufs))
        psum = ctx.enter_context(tc.tile_pool(name="psum", bufs=main_bufs, space="PSUM"))
        wops_p = ctx.enter_context(tc.tile_pool(name="wops", bufs=wo_bufs, space="PSUM"))

        # constants
        ident32 = consts.tile([128, 128], dt.float32)
        make_identity(nc, ident32)
        ident = consts.tile([128, 128], dt.bfloat16)
        nc.vector.tensor_copy(ident, ident32)
        ones32 = consts.tile([128, 128], dt.float32)
        nc.vector.memset(ones32, 1.0)
        ones = consts.tile([128, 128], dt.bfloat16)
        nc.vector.tensor_copy(ones, ones32)
        # PE warm-up: keep the PE busy through the initial DMA window so the
        # p-state ramp completes before real matmuls, and preload the Exp
        # activation table so the first attention batch doesn't pay it.
        warm_ps = psum.tile([128, 512], dt.float32, tag="ps", name="warm")
        for i in range(pe_warmup):
            nc.tensor.matmul(warm_ps[:, 0:128], lhsT=ident, rhs=ones,
                             start=True, stop=True)
        warm_sb = smalls.tile([128, 128], dt.float32, tag="warm_sb")
        nc.scalar.activation(warm_sb, warm_ps[:, 0:128],
                             mybir.ActivationFunctionType.Exp, scale=0.0)

        # weights + the whole hsT slab persist in SBUF (all bf16).
        # Few BIG multi-tile DMAs (HWDGE queue overhead is per-instruction).
        wq_all = wpool.tile([128, KT * GD], dt.bfloat16, tag="wq", name="wq_all")
        wk_all = wpool.tile([128, KT * D], dt.bfloat16, tag="wk", name="wk_all")
        wv_all = wpool.tile([128, KT * D], dt.bfloat16, tag="wv", name="wv_all")
        hs_all = wpool.tile([128, KT * S], dt.bfloat16, tag="hs", name="hs_all")
        wq_t = [wq_all[:, t * GD:(t + 1) * GD] for t in range(KT)]
        wk_t = [wk_all[:, t * D:(t + 1) * D] for t in range(KT)]
        wv_t = [wv_all[:, t * D:(t + 1) * D] for t in range(KT)]
        hs_t = [hs_all[:, t * S:(t + 1) * S] for t in range(KT)]
        mask_t = []
        for i in range(2):
            mt = consts.tile([128, 128], dt.bfloat16, tag=f"mask{i}", name=f"mask{i}")
            mask_t.append(mt)
        wo_all = wpool.tile([128, 4 * H], dt.bfloat16, tag="wo", name="wo_all")
        wo_t = [wo_all[:, ct * H:(ct + 1) * H] for ct in range(4)]

        # persistent activations
        qT = [big.tile([128, S], dt.bfloat16, tag=f"qT{h}", name=f"qT{h}") for h in range(GH)]
        kT = big.tile([128, S], dt.bfloat16, tag="kT")
        v = big.tile([128, S], dt.bfloat16, tag="v")  # [s%128, (sblk, d)]
        ohT = [big.tile([128, S], dt.bfloat16, tag=f"ohT{h}", name=f"ohT{h}") for h in range(GH)]

        # ---- DMA issue: first chunk's operands first, then the rest ----
        # wq as two halves (t 0-7, 8-15) so the first q-group starts sooner;
        # hs chunk DMAs carry all 16 t-tiles' 512-column slices in one instr.
        # SBUF-side DMA APs need the partition dim outermost; DRAM side is
        # rearranged to the same p-outer element order.
        def dma_hs_chunk(ch, t0, t1):
            nc.sync.dma_start(
                out=hs_all.rearrange("p (t s) -> p t s", t=KT)[:, t0:t1, ch * 512:(ch + 1) * 512],
                in_=hsT.rearrange("t p s -> p t s")[:, t0:t1, ch * 512:(ch + 1) * 512])

        wq_sb = wq_all.rearrange("p (t n) -> p t n", t=KT)
        wq_dr = wq.rearrange("(t p) n -> p t n", t=KT)
        first_pieces = [(0, 2), (2, 4), (4, 8), (8, 12), (12, 16)]
        for t0, t1 in first_pieces:
            nc.sync.dma_start(out=wq_sb[:, t0:t1], in_=wq_dr[:, t0:t1])
            dma_hs_chunk(0, t0, t1)
        nc.sync.dma_start(out=wk_all.rearrange("p (t n) -> p t n", t=KT),
                          in_=wk.rearrange("(t p) n -> p t n", t=KT))
        nc.sync.dma_start(out=wv_all.rearrange("p (t n) -> p t n", t=KT),
                          in_=wv.rearrange("(t p) n -> p t n", t=KT))
        for ch in range(1, 4):
            dma_hs_chunk(ch, 0, 16)
        for i in range(2):
            nc.sync.dma_start(out=mask_t[i], in_=masks[i])
        nc.sync.dma_start(out=wo_all.rearrange("p (t n) -> p t n", t=4),
                          in_=wo.rearrange("(t p) n -> p t n", t=4))

        # ---- Phase 1: projections per 256-wide half-chunk (3 PSUM banks each) ----
        # bank qps2: [h0 | h1] halves; qps2b: [h2 | h3]; bank kv: [k | v0 | v1]
        # NOTE: matmul start=True clears accumulation state for the WHOLE PSUM
        # bank, so co-resident groups in one bank must be issued contiguously
        # (a group fully closes before the next group's start): t-inner loops.
        # Chunk 0 runs t-major with one bank per q head so the PE can consume
        # quarter-granularity DMA arrivals without inter-group hazards.
        q_ps0 = [psum.tile([128, 512], dt.float32, tag="ps", name=f"q0_{h}")
                 for h in range(GH)]
        for t in range(KT):
            for h in range(GH):
                nc.tensor.matmul(q_ps0[h], lhsT=wq_t[t][:, h * 128:(h + 1) * 128],
                                 rhs=hs_t[t][:, 0:512],
                                 start=(t == 0), stop=(t == KT - 1))
        kv0_k = psum.tile([128, 512], dt.float32, tag="ps", name="k0")
        for t in range(KT):
            nc.tensor.matmul(kv0_k, lhsT=wk_t[t], rhs=hs_t[t][:, 0:512],
                             start=(t == 0), stop=(t == KT - 1))
        kv0_v = psum.tile([128, 512], dt.float32, tag="ps", name="v0")
        for j in range(4):
            for t in range(KT):
                nc.tensor.matmul(kv0_v[:, j * 128:(j + 1) * 128],
                                 lhsT=hs_t[t][:, j * 128:(j + 1) * 128],
                                 rhs=wv_t[t], start=(t == 0), stop=(t == KT - 1))
        for h in range(2):
            nc.vector.tensor_copy(qT[h][:, 0:512], q_ps0[h])
            nc.scalar.copy(qT[h + 2][:, 0:512], q_ps0[h + 2])
        nc.vector.tensor_copy(kT[:, 0:512], kv0_k)
        nc.vector.tensor_copy(v[:, 0:512], kv0_v)

        for hc in range(2, 8):
            s0 = hc * 256
            qps_a = psum.tile([128, 512], dt.float32, tag="ps", name=f"qa{hc}")
            qps_b = psum.tile([128, 512], dt.float32, tag="ps", name=f"qb{hc}")
            kv_ps = psum.tile([128, 512], dt.float32, tag="ps", name=f"kv{hc}")
            for h in range(2):
                for t in range(KT):
                    nc.tensor.matmul(qps_a[:, h * 256:(h + 1) * 256],
                                     lhsT=wq_t[t][:, h * 128:(h + 1) * 128],
                                     rhs=hs_t[t][:, s0:s0 + 256],
                                     start=(t == 0), stop=(t == KT - 1))
            for h in range(2):
                for t in range(KT):
                    nc.tensor.matmul(qps_b[:, h * 256:(h + 1) * 256],
                                     lhsT=wq_t[t][:, (h + 2) * 128:(h + 3) * 128],
                                     rhs=hs_t[t][:, s0:s0 + 256],
                                     start=(t == 0), stop=(t == KT - 1))
            for t in range(KT):
                nc.tensor.matmul(kv_ps[:, 0:256], lhsT=wk_t[t],
                                 rhs=hs_t[t][:, s0:s0 + 256],
                                 start=(t == 0), stop=(t == KT - 1))
            for j in range(2):
                for t in range(KT):
                    nc.tensor.matmul(kv_ps[:, 256 + j * 128:256 + (j + 1) * 128],
                                     lhsT=hs_t[t][:, s0 + j * 128:s0 + (j + 1) * 128],
                                     rhs=wv_t[t], start=(t == 0), stop=(t == KT - 1))
            for h in range(2):
                nc.vector.tensor_copy(qT[h][:, s0:s0 + 256],
                                      qps_a[:, h * 256:(h + 1) * 256])
                nc.scalar.copy(qT[h + 2][:, s0:s0 + 256],
                               qps_b[:, h * 256:(h + 1) * 256])
            nc.vector.tensor_copy(kT[:, s0:s0 + 256], kv_ps[:, 0:256])
            # v blocks 2*hc, 2*hc+1 -> v[:, blk*128:(blk+1)*128]
            nc.vector.tensor_copy(v[:, s0:s0 + 256], kv_ps[:, 256:512])

        if debug == 1:
            # dump projections: out[0..3]=qT, out[4]=kT, out[5]=v
            for e in range(4):
                for h in range(GH):
                    nc.sync.dma_start(out=out[h, e], in_=qT[h][:, e * 512:(e + 1) * 512])
                nc.sync.dma_start(out=out[4, e], in_=kT[:, e * 512:(e + 1) * 512])
                nc.sync.dma_start(out=out[5, e], in_=v[:, e * 512:(e + 1) * 512])

        # ---- Phase 2+3: banded attention (qb-outer) + Wo row-tiles ----
        # per (h, qb): kjs = [max(0, qb-8) .. qb]; score blocks [128k x 128q]
        # accumulated transposed; exp batches of <=4 blocks per PSUM bank.
        pending = []   # (avden, pts, pt, kj_list, first, last, h, qb)

        def flush_one():
            # av accumulates alone as the bank's open group; den is issued as
            # one contiguous group into the same bank only after av has closed
            # (a start=True clears accumulation bits bank-wide).
            avden, pts, pt, kjl, first, last, h, qb = pending.pop(0)
            n = len(kjl)
            for i, kj in enumerate(kjl):
                nc.tensor.matmul(avden[:, 0:128], lhsT=v[:, kj * 128:(kj + 1) * 128],
                                 rhs=pt[:, i * 128:(i + 1) * 128],
                                 start=(first and i == 0), stop=(last and i == n - 1))
            if last:
                nkj = sum(len(bk) for _, bk in pts)
                d = 0
                for ptt, bk in pts:
                    for i in range(len(bk)):
                        nc.tensor.matmul(avden[:, 128:256], lhsT=ones,
                                         rhs=ptt[:, i * 128:(i + 1) * 128],
                                         start=(d == 0), stop=(d == nkj - 1))
                        d += 1
                rcb = smalls.tile([128, 128], dt.float32, tag="rcb")
                with nc.allow_low_precision(reason="fp32 reciprocal, full precision"):
                    nc.vector.reciprocal(rcb, avden[:, 128:256])
                nc.vector.tensor_mul(ohT[h][:, qb * 128:(qb + 1) * 128],
                                     avden[:, 0:128], rcb)

        for qb in range(NB if debug != 1 else 0):
            for h in range(GH):
                kjs = list(range(max(0, qb - 8), qb + 1))
                avden = psum.tile([128, 512], dt.float32, tag="ps", name=f"ad{qb}_{h}")
                qs = qT[h][:, qb * 128:(qb + 1) * 128]
                pts = []
                for bi in range(0, len(kjs), 4):
                    bk = kjs[bi:bi + 4]
                    sps = psum.tile([128, 512], dt.float32, tag="ps")
                    for i, kj in enumerate(bk):
                        nc.tensor.matmul(sps[:, i * 128:(i + 1) * 128],
                                         lhsT=kT[:, kj * 128:(kj + 1) * 128],
                                         rhs=qs, start=True, stop=True)
                    pt = ptp.tile([128, 512], dt.bfloat16, tag="pt")
                    nc.scalar.activation(pt[:, :128 * len(bk)], sps[:, :128 * len(bk)],
                                         mybir.ActivationFunctionType.Exp, scale=SCALE)
                    # mask by zeroing exp weights (unmasked exp can't overflow:
                    # |score*scale| <= sqrt(128)*|q||k|*scale ~ O(12))
                    for i, kj in enumerate(bk):
                        mi = 0 if kj == qb else (1 if kj == qb - 8 else None)
                        if mi is not None:
                            nc.vector.tensor_mul(pt[:, i * 128:(i + 1) * 128],
                                                 pt[:, i * 128:(i + 1) * 128],
                                                 mask_t[mi])
                    pts.append((pt, bk))
                    pending.append((avden, pts, pt, bk, bi == 0, bi + 4 >= len(kjs), h, qb))
                    while len(pending) > pipe_depth:
                        flush_one()
            # Wo row-tile st=qb-1; first drain any pending work for that qb so
            # its divides are issued before the Wo matmuls read ohT
            if qb >= 1:
                while any(item[7] == qb - 1 for item in pending):
                    flush_one()
                emit_wo(nc, wops_p, outp, ohT, wo_t, out, qb - 1)
        while pending:
            flush_one()
        if debug != 1:
            emit_wo(nc, wops_p, outp, ohT, wo_t, out, NB - 1, split_dma=False)

    nc.compile()
    return nc


def emit_wo(nc, wops_p, outp, ohT, wo_t, out, st, split_dma=False):
    osb = outp.tile([128, 4 * 512], dt.bfloat16, tag="osb")
    for e in range(4):
        wops = wops_p.tile([128, 512], dt.float32, tag="wo")
        for ct in range(4):
            nc.tensor.matmul(wops, lhsT=ohT[ct][:, st * 128:(st + 1) * 128],
                             rhs=wo_t[ct][:, e * 512:(e + 1) * 512],
                             start=(ct == 0), stop=(ct == 3))
        nc.vector.tensor_copy(osb[:, e * 512:(e + 1) * 512], wops)
        if split_dma:
            nc.sync.dma_start(out=out[st, e], in_=osb[:, e * 512:(e + 1) * 512])
    if not split_dma:
        nc.sync.dma_start(out=out[st].rearrange("e p n -> p e n"), in_=osb)


def _build_masks():
    kk = np.arange(128)[:, None]
    qq = np.arange(128)[None, :]
    diag = (kk <= qq).astype(np.float32)   # causal within diag block
    edge = (kk >= qq).astype(np.float32)   # window lower edge
    return np.stack([diag, edge])


def kernel(hidden_states, Wq, Wk, Wv, Wo):
    global _nc_cache
    if _nc_cache is None:
        _nc_cache = _build_nc()
    nc = _nc_cache

    bf16 = dt.np(dt.bfloat16)
    masks = _build_masks().astype(bf16)
    hsT = []
    for b in range(B):
        ht = np.ascontiguousarray(hidden_states[b].T.astype(bf16))    # [H, S]
        hsT.append(np.ascontiguousarray(ht.reshape(KT, 128, S)))
    in_maps = []
    for b in range(B):
        for gi in range(KV_HEADS):
            in_maps.append({
                "hsT": hsT[b],
                "wq": np.ascontiguousarray(Wq[:, gi * GD:(gi + 1) * GD].astype(bf16)),
                "wk": np.ascontiguousarray(Wk[:, gi * D:(gi + 1) * D].astype(bf16)),
                "wv": np.ascontiguousarray(Wv[:, gi * D:(gi + 1) * D].astype(bf16)),
                "wo": np.ascontiguousarray(Wo[gi * GD:(gi + 1) * GD, :].astype(bf16)),
                "masks": masks,
            })
    res = run_bass_kernel_spmd(nc, in_maps, list(range(8)))
    out = np.zeros((B, S, H), np.float32)
    for b in range(B):
        acc = None
        for gi in range(KV_HEADS):
            o = res.results[b * KV_HEADS + gi]["out"].astype(np.float32)
            acc = o if acc is None else acc + o
        out[b] = acc.transpose(0, 2, 1, 3).reshape(S, H)              # [16,4,128,512] -> [S,H]
    return out
